# revision 16
# baseline (speedup 1.0000x reference)
"""GATv2FeatureExtractor Trainium2 kernel (8 NeuronCores, edge-parallel by dst).

v2 design
---------
Edges sorted by destination, sharded into 8 contiguous dst ranges (6250 nodes
per core); local node order is degree-balanced so fixed node-windows (55 nodes
for GAT layer 1, 119 for layer 2) carry near-equal edge counts.  The weighted
message scatter is a one-hot matmul into a PSUM window (no cross-core
reduction).  Two AllGathers: h after a *sharded* MLP encoder, and the layer-2
source table (h1 @ Wl2) between the GAT layers.

Key mechanics vs v1:
- Per-window batched indirect gather (one SWDGE descriptor-gen call per
  window instead of per 128-edge chunk: 994ns fixed overhead amortized).
- alpha via the relu decomposition  att . LReLU(s) = 0.2*(att.s) + 0.8*(att.relu(s)):
  the 0.2 linear term rides as 4 extra matmul columns; the relu term is ONE
  fused (0 max s)*att vector op + ONE segmented tensor_reduce per chunk.
- Payload scaling via a single broadcast tensor_tensor per chunk (ex
  broadcast over each head's 65-column block).
- One-hot rows built with a single broadcast is_equal per window.
- Per-window chunk counts (variable k) instead of a global max.
- Edge attrs DMA'd contiguously then strided into place on-chip (128 fat
  descriptors per window instead of 128*k tiny ones).

Compute in fp16 (f32 PSUM accumulation); the layer-2 exp bias column is
carried as an fp16 hi/lo pair.
"""

import os
import sys

import numpy as np

if os.path.isdir("/opt/trn_rl_repo") and "/opt/trn_rl_repo" not in sys.path:
    sys.path.insert(0, "/opt/trn_rl_repo")

import concourse.bacc as bacc
import concourse.bass as bass
import concourse.mybir as mybir
import concourse.tile as tile
from concourse.bass import IndirectOffsetOnAxis, broadcast_tensor_aps
from concourse.bass_utils import run_bass_kernel_spmd

F32 = mybir.dt.float32
BF16 = mybir.dt.float16  # 16-bit compute dtype (fp16: better mantissa, same speed)
I32 = mybir.dt.int32
AF = mybir.ActivationFunctionType
ALU = mybir.AluOpType

NCORES = 8
P = 128

F_IN, ED, HID, H, OUT = 32, 8, 64, 4, 256
HC = H * HID  # 256
XW = 260      # xl2 table width: 256 payload + tl (f32 in 2 slots) + ones + pad


def _bf(a):
    return np.asarray(a, np.float32).astype(np.float16)


def _ceil_div(a, b):
    return -(-a // b)


# ----------------------------------------------------------------------------
# host-side preprocessing
# ----------------------------------------------------------------------------

def _balanced_perms(deg, n, npc, span):
    """Per-core degree-balanced local permutation for `span`-node windows."""
    perms, invs = [], []
    for c in range(NCORES):
        d = deg[c * npc:(c + 1) * npc]
        order = np.argsort(-d, kind="stable")
        rows = np.arange(npc)
        seq = np.lexsort((rows // span, rows % span))
        perm = np.empty(npc, np.int64)
        perm[seq] = order
        inv = np.empty(npc, np.int64)
        inv[perm] = np.arange(npc)
        perms.append(perm)
        invs.append(inv)
    return perms, invs


def _pack_var(src_row, dstl, winid, ea_e, nw, kws, offs, sumk):
    """Pack one core's edges into variable-k chunk-major arrays [P, sumk]."""
    src_f = np.zeros((P, sumk), np.int32)
    dst_f = np.full((P, sumk), -1.0, np.float32)
    ea_f = np.zeros((P, sumk, ED), np.float32)
    order = np.argsort(winid, kind="stable")
    bounds = np.searchsorted(winid[order], np.arange(nw + 1))
    for w in range(nw):
        a, b = bounds[w], bounds[w + 1]
        m = b - a
        if m == 0:
            continue
        sel = order[a:b]
        jj = np.arange(m)
        lane, chunk = jj % P, offs[w] + jj // P
        src_f[lane, chunk] = src_row[sel]
        dst_f[lane, chunk] = dstl[sel]
        ea_f[lane, chunk] = ea_e[sel]
    return src_f, dst_f, _bf(ea_f.reshape(P, sumk * ED))


def _prep_host(inputs, n, e, npc, w1=55, w2=119):
    x = np.asarray(inputs["x"], np.float32)
    ei = np.asarray(inputs["edge_index"])
    ea = np.asarray(inputs["edge_attr"], np.float32)
    src = ei[0].astype(np.int64)
    dst = ei[1].astype(np.int64)

    deg = np.bincount(dst, minlength=n)
    perms, invs = _balanced_perms(deg, n, npc, w1)
    nw1 = _ceil_div(npc, w1)
    nw2 = _ceil_div(npc, w2)
    inv_all = np.concatenate(invs)
    grow = (src // npc) * npc + inv_all[src]  # global balanced row of src

    owner = dst // npc
    core_pack = []
    cnt1 = np.zeros((NCORES, nw1), np.int64)
    cnt2 = np.zeros((NCORES, nw2), np.int64)
    for c in range(NCORES):
        es = np.where(owner == c)[0]
        r = invs[c][dst[es] - c * npc]
        w1id = r // w1
        w2id = r // w2
        np.add.at(cnt1[c], w1id, 1)
        np.add.at(cnt2[c], w2id, 1)
        core_pack.append((es, r, w1id, w2id))
    kws1 = np.maximum(1, _ceil_div(cnt1.max(0), P)).astype(np.int64)
    kws2 = np.maximum(1, _ceil_div(cnt2.max(0), P)).astype(np.int64)
    offs1 = np.concatenate([[0], np.cumsum(kws1)])
    offs2 = np.concatenate([[0], np.cumsum(kws2)])
    sumk1, sumk2 = int(offs1[-1]), int(offs2[-1])

    packed1, packed2 = [], []
    for c in range(NCORES):
        es, r, w1id, w2id = core_pack[c]
        g = grow[es]
        packed1.append(_pack_var(g, r - w1id * w1, w1id, ea[es], nw1, kws1, offs1, sumk1))
        packed2.append(_pack_var(g, r - w2id * w2, w2id, ea[es], nw2, kws2, offs2, sumk2))

    # --- weights ---
    gi = lambda k: np.asarray(inputs[k], np.float32)
    W1, b1, W2, b2 = gi("W1"), gi("b1"), gi("W2"), gi("b2")
    Wl1, bl1, Wr1, br1 = gi("Wl1"), gi("bl1"), gi("Wr1"), gi("br1")
    We1, att1, bias1 = gi("We1"), gi("att1"), gi("bias1")
    Wl2, bl2, Wr2, br2 = gi("Wl2"), gi("bl2"), gi("Wr2"), gi("br2")
    We2, att2, bias2 = gi("We2"), gi("att2"), gi("bias2")

    consts = {}
    consts["ident"] = _bf(np.eye(P, dtype=np.float32))
    consts["ones1"] = _bf(np.ones((1, P), np.float32))
    consts["mlp1"] = _bf(np.concatenate([W1, b1[None, :]], 0))
    consts["mlp2"] = _bf(np.concatenate([W2, b2[None, :]], 0))

    amat1 = np.zeros((HC, H), np.float32)
    for h in range(H):
        amat1[h * HID:(h + 1) * HID, h] = att1[h]

    def _aug1(m):  # [_, 256] -> [_, 260] with 0.2*linear columns
        return np.concatenate([m, 0.2 * (m @ amat1)], 1)

    brow1 = (bl1 + br1)[None, :]
    rc1 = np.zeros((P, HC + H), np.float32)
    rc1[0:HID] = _aug1(Wl1)
    rc1[HID:HID + ED] = _aug1(We1)
    rc1[127] = _aug1(brow1)[0]
    consts["rc1"] = _bf(rc1)
    consts["wr1aug"] = _bf(_aug1(Wr1))
    # payload: per head [64 cols of Wl1 | ones col]
    rc2 = np.zeros((P, HC + H), np.float32)
    pb = bl1 + bias1
    for h in range(H):
        rc2[0:HID, 65 * h:65 * h + HID] = Wl1[:, HID * h:HID * (h + 1)]
        rc2[127, 65 * h:65 * h + HID] = pb[HID * h:HID * (h + 1)]
        rc2[127, 65 * h + HID] = 1.0
    consts["rc2"] = _bf(rc2)
    consts["attw1"] = _bf(np.tile(0.8 * att1.reshape(1, HC), (P, 1)))

    arow2 = att2.reshape(HC)
    a2m = arow2[:, None]

    def _aug2(m):  # [_, 256] -> [_, 257]
        return np.concatenate([m, 0.2 * (m @ a2m)], 1)

    brow2 = (br2 - bias2)[None, :]
    rc21 = np.zeros((P, HC + 1), np.float32)
    rc21[0:ED] = _aug2(We2)
    rc21[127] = _aug2(brow2)[0]
    consts["rc21"] = _bf(rc21)
    wr2 = _aug2(Wr2)
    consts["wr2a"], consts["wr2b"] = _bf(wr2[0:P]), _bf(wr2[P:2 * P])
    wl2 = _aug2(Wl2)
    consts["wl2a"], consts["wl2b"] = _bf(wl2[0:P]), _bf(wl2[P:2 * P])
    xb = (bl2 + bias2)[None, :]
    x2b = _aug2(xb)
    x2b[0, HC] -= 4.0  # exp overflow guard rides the tl column
    consts["xl2bias"] = _bf(x2b)
    consts["attw2"] = _bf(np.tile(0.8 * arow2[None, :], (P, 1)))

    nch0 = _ceil_div(npc, 512)
    npcpad = nch0 * 512

    in_maps = []
    for c in range(NCORES):
        lo = c * npc
        xt = np.zeros((F_IN + 1, npcpad), np.float16)
        xt[0:F_IN, :npc] = _bf(x[lo + perms[c]].T)
        xt[F_IN, :npc] = 1.0
        m = dict(consts)
        m["xt"] = xt
        m["src1"], m["dstl1"], m["ea1"] = packed1[c]
        m["src2"], m["dstl2"], m["ea2"] = packed2[c]
        in_maps.append(m)

    meta = dict(n=n, npc=npc, npcpad=npcpad, nch0=nch0,
                w1=w1, nw1=nw1, kws1=[int(v) for v in kws1], offs1=[int(v) for v in offs1],
                w2=w2, nw2=nw2, kws2=[int(v) for v in kws2], offs2=[int(v) for v in offs2],
                sumk1=sumk1, sumk2=sumk2,
                k1max=int(kws1.max()), k2max=int(kws2.max()))
    return meta, in_maps, perms


# ----------------------------------------------------------------------------
# device program
# ----------------------------------------------------------------------------

def _build_nc(meta, debug=False):
    n, npc, npcpad, nch0 = meta["n"], meta["npc"], meta["npcpad"], meta["nch0"]
    w1, nw1, kws1, offs1 = meta["w1"], meta["nw1"], meta["kws1"], meta["offs1"]
    w2, nw2, kws2, offs2 = meta["w2"], meta["nw2"], meta["kws2"], meta["offs2"]
    sumk1, sumk2 = meta["sumk1"], meta["sumk2"]
    k1max, k2max = meta["k1max"], meta["k2max"]

    nc = bacc.Bacc("TRN2", target_bir_lowering=False, num_devices=NCORES)

    def din(name, shape, dtype=BF16):
        return nc.dram_tensor(name, shape, dtype, kind="ExternalInput")

    ident_d = din("ident", [P, P])
    ones1_d = din("ones1", [1, P])
    mlp1_d = din("mlp1", [F_IN + 1, HID])
    mlp2_d = din("mlp2", [HID + 1, HID])
    rc1_d = din("rc1", [P, HC + H])
    rc2_d = din("rc2", [P, HC + H])
    wr1aug_d = din("wr1aug", [HID, HC + H])
    attw1_d = din("attw1", [P, HC])
    rc21_d = din("rc21", [P, HC + 1])
    wr2a_d = din("wr2a", [P, HC + 1]); wr2b_d = din("wr2b", [P, HC + 1])
    wl2a_d = din("wl2a", [P, HC + 1]); wl2b_d = din("wl2b", [P, HC + 1])
    xl2bias_d = din("xl2bias", [1, HC + 1])
    attw2_d = din("attw2", [P, HC])
    xt_d = din("xt", [F_IN + 1, npcpad])
    src1_d = din("src1", [P, sumk1], I32)
    dstl1_d = din("dstl1", [P, sumk1], F32)
    ea1_d = din("ea1", [P, sumk1 * ED])
    src2_d = din("src2", [P, sumk2], I32)
    dstl2_d = din("dstl2", [P, sumk2], F32)
    ea2_d = din("ea2", [P, sumk2 * ED])
    out_d = nc.dram_tensor("out", [npc, HC], F32, kind="ExternalOutput")
    if debug:
        dbg_h = nc.dram_tensor("dbg_h", [n, HID], BF16, kind="ExternalOutput")
        dbg_h1 = nc.dram_tensor("dbg_h1", [npc, HC], BF16, kind="ExternalOutput")
        dbg_xf = nc.dram_tensor("dbg_xf", [n, XW], BF16, kind="ExternalOutput")

    with tile.TileContext(nc) as tc:
        with (
            tc.tile_pool(name="dram", bufs=1, space="DRAM") as dram,
            tc.tile_pool(name="const", bufs=1) as cpool,
        ):
            hloc = dram.tile([npcpad, HID], BF16)
            h_full = dram.tile([n, HID], BF16)
            h1loc = dram.tile([npc, HC], BF16)
            xl2loc = dram.tile([npc, XW], BF16)
            xl2full = dram.tile([n, XW], BF16)

            def cload(name, shape, src_d, dt=BF16):
                t = cpool.tile(shape, dt, tag=name)
                nc.sync.dma_start(t[:], src_d[:, :])
                return t

            ident = cload("ident", [P, P], ident_d)
            ones1 = cload("ones1", [1, P], ones1_d)
            mlp1 = cload("mlp1", [F_IN + 1, HID], mlp1_d)
            mlp2 = cload("mlp2", [HID + 1, HID], mlp2_d)
            rc1 = cload("rc1", [P, HC + H], rc1_d)
            rc2 = cload("rc2", [P, HC + H], rc2_d)
            wr1aug = cload("wr1aug", [HID, HC + H], wr1aug_d)
            attw1 = cload("attw1", [P, HC], attw1_d)
            rc21 = cload("rc21", [P, HC + 1], rc21_d)
            wr2a = cload("wr2a", [P, HC + 1], wr2a_d)
            wr2b = cload("wr2b", [P, HC + 1], wr2b_d)
            wl2a = cload("wl2a", [P, HC + 1], wl2a_d)
            wl2b = cload("wl2b", [P, HC + 1], wl2b_d)
            xl2bias = cload("xl2bias", [1, HC + 1], xl2bias_d)
            attw2 = cload("attw2", [P, HC], attw2_d)
            iotaF = cpool.tile([P, w2], F32, tag="iotaF")
            nc.gpsimd.iota(iotaF[:], pattern=[[1, w2]], base=0,
                           channel_multiplier=0,
                           allow_small_or_imprecise_dtypes=True)
            neg4 = cpool.tile([P, 1], F32, tag="neg4")
            nc.vector.memset(neg4[:], -4.0)

            # ---------------- phase 0: sharded MLP encoder -> hloc -----------
            with (
                tc.tile_pool(name="mlp", bufs=2) as mpool,
                tc.tile_pool(name="mps", bufs=2, space="PSUM") as mps,
            ):
                for i in range(nch0):
                    sl = slice(i * 512, (i + 1) * 512)
                    rx = mpool.tile([F_IN + 1, 512], BF16, tag="rx")
                    nc.sync.dma_start(rx[:], xt_d[:, sl])
                    p1 = mps.tile([HID, 512], F32, tag="p1")
                    nc.tensor.matmul(p1[:], lhsT=mlp1[:], rhs=rx[:], start=True, stop=True)
                    ht = mpool.tile([HID + 1, 512], BF16, tag="ht")
                    nc.scalar.activation(ht[0:HID, :], p1[:], AF.Relu)
                    nc.vector.memset(ht[HID:HID + 1, :], 1.0)
                    p2 = mps.tile([HID, 512], F32, tag="p2")
                    nc.tensor.matmul(p2[:], lhsT=mlp2[:], rhs=ht[:], start=True, stop=True)
                    h2 = mpool.tile([HID, 512], BF16, tag="h2")
                    nc.scalar.activation(h2[:], p2[:], AF.Relu)
                    hrow = mpool.tile([P, 4, HID], BF16, tag="hrow")
                    for j in range(4):
                        pt = mps.tile([P, HID], BF16, tag="pt")
                        nc.tensor.transpose(pt[:], h2[:, j * P:(j + 1) * P],
                                            ident[0:HID, 0:HID])
                        nc.scalar.activation(hrow[:, j, :], pt[:], AF.Copy)
                    nc.sync.dma_start(
                        hloc[sl, :].rearrange("(j p) d -> p j d", p=P), hrow[:])

            # ---------------- allgather h ------------------------------------
            nc.gpsimd.collective_compute(
                "AllGather", ALU.bypass,
                replica_groups=[list(range(NCORES))],
                ins=[hloc[0:npc, :]], outs=[h_full[:]])

            # ---------------- phase 1: GAT layer 1 ---------------------------
            rc1w_a = cpool.tile([P, HC + H], BF16, tag="rc1w_a")
            rc1w_b = cpool.tile([P, HC + H], BF16, tag="rc1w_b")
            rc21w_a = cpool.tile([P, HC + 1], BF16, tag="rc21w_a")
            rc21w_b = cpool.tile([P, HC + 1], BF16, tag="rc21w_b")
            rc1ws = [rc1w_a, rc1w_b]
            rc21ws = [rc21w_a, rc21w_b]
            with (
                tc.tile_pool(name="w1p", bufs=2) as wpool,
                tc.tile_pool(name="e1p", bufs=3) as epool,
                tc.tile_pool(name="kp1", bufs=3) as kpool,
                tc.tile_pool(name="ps_s", bufs=2, space="PSUM") as ps_s,
                tc.tile_pool(name="ps_x", bufs=2, space="PSUM") as ps_x,
                tc.tile_pool(name="ps_t", bufs=2, space="PSUM") as ps_t,
                tc.tile_pool(name="ps_o", bufs=1, space="PSUM") as ps_o,
                tc.tile_pool(name="ps_p", bufs=1, space="PSUM") as ps_p,
            ):
                for w in range(nw1):
                    span = min(w1, npc - w * w1)
                    nb = w * w1
                    kw = kws1[w]
                    off = offs1[w]

                    # window prep: xr for this window's dst nodes
                    hwin = wpool.tile([64, HID], BF16, tag="hwin")
                    nc.sync.dma_start(hwin[0:span, :], hloc[nb:nb + span, :])
                    ptw = ps_t.tile([P, P], BF16, tag="ptp")
                    nc.tensor.transpose(ptw[0:HID, 0:span], hwin[0:span, :],
                                        ident[0:span, 0:span])
                    hwT = wpool.tile([HID, 64], BF16, tag="hwT")
                    nc.scalar.activation(hwT[:, 0:span], ptw[0:HID, 0:span], AF.Copy)
                    pxr = ps_p.tile([64, HC + H], F32, tag="pxp")
                    nc.tensor.matmul(pxr[0:span, :], lhsT=hwT[:, 0:span],
                                     rhs=wr1aug[:], start=True, stop=True)
                    rc1w = rc1ws[w % 2]
                    if w < 2:
                        nc.sync.dma_start(rc1w[:], rc1[:])
                    xrw = wpool.tile([64, HC + H], BF16, tag="xrw")
                    nc.scalar.activation(xrw[0:span, :], pxr[0:span, :], AF.Copy)
                    nc.sync.dma_start(rc1w[72:72 + span, :], xrw[0:span, :])

                    # per-window edge data
                    srcw = epool.tile([P, k1max], I32, tag="srcw")
                    nc.sync.dma_start(srcw[:, 0:kw], src1_d[:, off:off + kw])
                    dstw = wpool.tile([P, k1max], F32, tag="dstw")
                    nc.sync.dma_start(dstw[:, 0:kw], dstl1_d[:, off:off + kw])
                    pre = epool.tile([P, k1max, P], BF16, tag="pre")
                    for c in range(kw):
                        nc.gpsimd.indirect_dma_start(
                            out=pre[:, c, 0:HID], out_offset=None,
                            in_=h_full[:, :],
                            in_offset=IndirectOffsetOnAxis(ap=srcw[:, c:c + 1], axis=0))
                    nc.sync.dma_start(
                        pre[:, 0:kw, HID:HID + ED],
                        ea1_d[:, off * ED:(off + kw) * ED].rearrange(
                            "p (k d) -> p k d", d=ED))
                    nc.vector.memset(pre[:, 0:kw, 127:P], 1.0)
                    i0, i1 = broadcast_tensor_aps(
                        iotaF[:, None, 0:w1], dstw[:, 0:kw, None])
                    nc.vector.tensor_tensor(
                        out=pre[:, 0:kw, 72:127], in0=i0, in1=i1, op=ALU.is_equal)
                    stk = epool.tile([P, k1max, P], BF16, tag="stk")
                    for c in range(kw):
                        ptc = ps_t.tile([P, P], BF16, tag="ptp")
                        nc.tensor.transpose(ptc[:], pre[:, c, :], ident[:])
                        nc.scalar.activation(stk[:, c, :], ptc[:], AF.Copy)

                    pout = ps_o.tile([64, HC + H], F32, tag="pout")
                    for c in range(kw):
                        pss = ps_s.tile([P, HC + H], F32, tag="pss")
                        nc.tensor.matmul(pss[:], lhsT=stk[:, c, :], rhs=rc1w[:],
                                         start=True, stop=True)
                        psx = ps_x.tile([P, HC + H], F32, tag="psx")
                        nc.tensor.matmul(psx[:], lhsT=stk[:, c, :], rhs=rc2[:],
                                         start=True, stop=True)
                        m = kpool.tile([P, HC], BF16, tag="m")
                        nc.vector.scalar_tensor_tensor(
                            out=m[:], in0=pss[:, 0:HC], scalar=0.0,
                            in1=attw1[:], op0=ALU.max, op1=ALU.mult)
                        alph = kpool.tile([P, H], F32, tag="alph")
                        nc.vector.tensor_reduce(
                            out=alph[:, :, None],
                            in_=m[:, :].rearrange("p (h x) -> p h x", x=HID),
                            axis=mybir.AxisListType.X, op=ALU.add)
                        alph2 = kpool.tile([P, H], F32, tag="alph2")
                        nc.vector.tensor_tensor(
                            out=alph2[:], in0=alph[:], in1=pss[:, HC:HC + H],
                            op=ALU.add)
                        ex = kpool.tile([P, H], F32, tag="ex")
                        nc.scalar.activation(ex[:], alph2[:], AF.Exp, bias=neg4[:])
                        pay = kpool.tile([P, HC + H], BF16, tag="pay")
                        x0, x1 = broadcast_tensor_aps(
                            psx[:, :].rearrange("p (h x) -> p h x", x=65),
                            ex[:, :, None])
                        nc.vector.tensor_tensor(
                            out=pay[:, :].rearrange("p (h x) -> p h x", x=65),
                            in0=x0, in1=x1, op=ALU.mult)
                        nc.tensor.matmul(pout[0:span, :],
                                         lhsT=pre[:, c, 72:72 + span],
                                         rhs=pay[:], start=(c == 0),
                                         stop=(c == kw - 1))

                    # normalize + relu -> h1 window; prep xl2 rows
                    pog = pout[0:span, :].rearrange("p (h x) -> p h x", x=65)
                    deng = wpool.tile([64, H], F32, tag="deng")
                    nc.vector.tensor_scalar(
                        out=deng[0:span, :, None], in0=pog[:, :, 64:65],
                        scalar1=1e-30, scalar2=None, op0=ALU.max)
                    rden = wpool.tile([64, H], F32, tag="rden")
                    nc.vector.reciprocal(rden[0:span, :], deng[0:span, :])
                    h1w = wpool.tile([64, HC], BF16, tag="h1w")
                    r0, r1 = broadcast_tensor_aps(
                        pog[:, :, 0:64], rden[0:span, :, None])
                    nc.vector.scalar_tensor_tensor(
                        out=h1w[0:span, :].rearrange("p (h x) -> p h x", x=HID),
                        in0=r0, scalar=0.0, in1=r1, op0=ALU.max, op1=ALU.mult)
                    nc.sync.dma_start(h1loc[nb:nb + span, :], h1w[0:span, :])

                    h1T = wpool.tile([P, 2, 64], BF16, tag="h1T")
                    for j in range(2):
                        ptj = ps_t.tile([P, P], BF16, tag="ptp")
                        nc.tensor.transpose(ptj[:, 0:span],
                                            h1w[0:span, j * P:(j + 1) * P],
                                            ident[0:span, 0:span])
                        nc.scalar.activation(h1T[:, j, 0:span], ptj[:, 0:span],
                                             AF.Copy)
                    pxl2 = ps_p.tile([64, HC + H], F32, tag="pxp")
                    nc.tensor.matmul(pxl2[0:span, 0:HC + 1], lhsT=h1T[:, 0, 0:span],
                                     rhs=wl2a[:], start=True, stop=False)
                    nc.tensor.matmul(pxl2[0:span, 0:HC + 1], lhsT=h1T[:, 1, 0:span],
                                     rhs=wl2b[:], start=False, stop=False)
                    nc.tensor.matmul(pxl2[0:span, 0:HC + 1], lhsT=ones1[:, 0:span],
                                     rhs=xl2bias[:], start=False, stop=True)
                    xl2w = wpool.tile([64, XW], BF16, tag="xl2w")
                    nc.scalar.activation(xl2w[0:span, 0:HC], pxl2[0:span, 0:HC],
                                         AF.Copy)
                    nc.vector.tensor_copy(
                        xl2w[0:span, HC:HC + 2].bitcast(F32),
                        pxl2[0:span, HC:HC + 1])
                    nc.vector.memset(xl2w[0:span, HC + 2:HC + 3], 1.0)
                    nc.vector.memset(xl2w[0:span, HC + 3:XW], 0.0)
                    nc.sync.dma_start(xl2loc[nb:nb + span, :], xl2w[0:span, :])

            # ---------------- phase 2: allgather xl2 table -------------------
            nc.gpsimd.collective_compute(
                "AllGather", ALU.bypass,
                replica_groups=[list(range(NCORES))],
                ins=[xl2loc[:]], outs=[xl2full[:]])

            # ---------------- phase 3: GAT layer 2 ---------------------------
            with (
                tc.tile_pool(name="w2p", bufs=2) as wpool,
                tc.tile_pool(name="e2p", bufs=3) as epool,
                tc.tile_pool(name="kp2", bufs=3) as kpool,
                tc.tile_pool(name="ps2_s", bufs=2, space="PSUM") as ps2_s,
                tc.tile_pool(name="ps2_t", bufs=2, space="PSUM") as ps2_t,
                tc.tile_pool(name="ps2_o", bufs=1, space="PSUM") as ps2_o,
                tc.tile_pool(name="ps2_p", bufs=1, space="PSUM") as ps2_p,
            ):
                for w in range(nw2):
                    span = min(w2, npc - w * w2)
                    nb = w * w2
                    kw = kws2[w]
                    off = offs2[w]
                    ng = _ceil_div(kw, 2)

                    h1r = wpool.tile([P, HC], BF16, tag="h1r")
                    nc.sync.dma_start(h1r[0:span, :], h1loc[nb:nb + span, :])
                    h1rT = wpool.tile([P, 2, P], BF16, tag="h1rT")
                    for j in range(2):
                        ptj = ps2_t.tile([P, P], BF16, tag="ptp2")
                        nc.tensor.transpose(ptj[:, 0:span],
                                            h1r[0:span, j * P:(j + 1) * P],
                                            ident[0:span, 0:span])
                        nc.scalar.activation(h1rT[:, j, 0:span], ptj[:, 0:span],
                                             AF.Copy)
                    pxr2 = ps2_p.tile([P, HC + 1], F32, tag="pxr2")
                    nc.tensor.matmul(pxr2[0:span, :], lhsT=h1rT[:, 0, 0:span],
                                     rhs=wr2a[:], start=True, stop=False)
                    nc.tensor.matmul(pxr2[0:span, :], lhsT=h1rT[:, 1, 0:span],
                                     rhs=wr2b[:], start=False, stop=True)
                    rc21w = rc21ws[w % 2]
                    if w < 2:
                        nc.sync.dma_start(rc21w[:], rc21[:])
                    xrw2 = wpool.tile([P, HC + 1], BF16, tag="xrw2")
                    nc.scalar.activation(xrw2[0:span, :], pxr2[0:span, :], AF.Copy)
                    nc.sync.dma_start(rc21w[ED:ED + span, :], xrw2[0:span, :])

                    srcw2 = epool.tile([P, k2max], I32, tag="srcw2")
                    nc.sync.dma_start(srcw2[:, 0:kw], src2_d[:, off:off + kw])
                    xg = epool.tile([P, k2max, XW], BF16, tag="xg")
                    for c in range(kw):
                        nc.gpsimd.indirect_dma_start(
                            out=xg[:, c, :], out_offset=None,
                            in_=xl2full[:, :],
                            in_offset=IndirectOffsetOnAxis(ap=srcw2[:, c:c + 1], axis=0))
                    dstw2 = wpool.tile([P, k2max], F32, tag="dstw2")
                    nc.sync.dma_start(dstw2[:, 0:kw], dstl2_d[:, off:off + kw])
                    pre2 = epool.tile([P, k2max, P], BF16, tag="pre2")
                    nc.sync.dma_start(
                        pre2[:, 0:kw, 0:ED],
                        ea2_d[:, off * ED:(off + kw) * ED].rearrange(
                            "p (k d) -> p k d", d=ED))
                    nc.vector.memset(pre2[:, 0:kw, 127:P], 1.0)
                    i0, i1 = broadcast_tensor_aps(
                        iotaF[:, None, 0:w2], dstw2[:, 0:kw, None])
                    nc.vector.tensor_tensor(
                        out=pre2[:, 0:kw, ED:ED + w2], in0=i0, in1=i1,
                        op=ALU.is_equal)
                    stk2 = epool.tile([P, k2max, P], BF16, tag="stk2")
                    for c in range(kw):
                        ptc = ps2_t.tile([P, P], BF16, tag="ptp2")
                        nc.tensor.transpose(ptc[:], pre2[:, c, :], ident[:])
                        nc.scalar.activation(stk2[:, c, :], ptc[:], AF.Copy)

                    pout2 = ps2_o.tile([P, HC + 3], F32, tag="pout2")
                    for g in range(ng):
                        c0 = 2 * g
                        gw = min(2, kw - c0)
                        pss2 = ps2_s.tile([P, 2, 512], F32, tag="pss2")
                        for j in range(gw):
                            nc.tensor.matmul(
                                pss2[:, j, 0:HC + 1], lhsT=stk2[:, c0 + j, :],
                                rhs=rc21w[:], start=True, stop=True)
                        s2 = kpool.tile([P, 2, HC], BF16, tag="s2")
                        nc.vector.tensor_tensor(
                            out=s2[:, 0:gw, :], in0=pss2[:, 0:gw, 0:HC],
                            in1=xg[:, c0:c0 + gw, 0:HC], op=ALU.add)
                        m2 = kpool.tile([P, 2, HC], BF16, tag="m2")
                        a0, a1 = broadcast_tensor_aps(
                            s2[:, 0:gw, :], attw2[:, None, :])
                        nc.vector.scalar_tensor_tensor(
                            out=m2[:, 0:gw, :], in0=a0, scalar=0.0,
                            in1=a1, op0=ALU.max, op1=ALU.mult)
                        al2 = kpool.tile([P, 2], F32, tag="al2")
                        nc.vector.tensor_reduce(
                            out=al2[:, 0:gw, None], in_=m2[:, 0:gw, :],
                            axis=mybir.AxisListType.X, op=ALU.add)
                        al2b = kpool.tile([P, 2], F32, tag="al2b")
                        nc.vector.tensor_tensor(
                            out=al2b[:, 0:gw, None], in0=al2[:, 0:gw, None],
                            in1=pss2[:, 0:gw, HC:HC + 1], op=ALU.add)
                        al2c = kpool.tile([P, 2], F32, tag="al2c")
                        nc.vector.tensor_tensor(
                            out=al2c[:, 0:gw, None], in0=al2b[:, 0:gw, None],
                            in1=xg[:, c0:c0 + gw, HC:HC + 2].bitcast(F32),
                            op=ALU.add)
                        ex2 = kpool.tile([P, 2], F32, tag="ex2")
                        nc.scalar.activation(ex2[:, 0:gw], al2c[:, 0:gw], AF.Exp)
                        pay2 = kpool.tile([P, 2, HC + 3], BF16, tag="pay2")
                        for j in range(gw):
                            nc.vector.tensor_scalar(
                                out=pay2[:, j, :], in0=xg[:, c0 + j, 0:HC + 3],
                                scalar1=ex2[:, j:j + 1], scalar2=None,
                                op0=ALU.mult)
                        for j in range(gw):
                            c = c0 + j
                            nc.tensor.matmul(
                                pout2[0:span, :], lhsT=pre2[:, c, ED:ED + span],
                                rhs=pay2[:, j, :], start=(c == 0),
                                stop=(c == kw - 1))

                    deng2 = wpool.tile([P, 1], F32, tag="deng2")
                    nc.vector.tensor_scalar(
                        out=deng2[0:span, :], in0=pout2[0:span, HC + 2:HC + 3],
                        scalar1=1e-30, scalar2=None, op0=ALU.max)
                    rden2 = wpool.tile([P, 1], F32, tag="rden2")
                    nc.vector.reciprocal(rden2[0:span, :], deng2[0:span, :])
                    outw = wpool.tile([P, HC], F32, tag="outw")
                    b0, b1 = broadcast_tensor_aps(
                        pout2[0:span, 0:HC], rden2[0:span, :])
                    nc.vector.scalar_tensor_tensor(
                        out=outw[0:span, :], in0=b0, scalar=0.0,
                        in1=b1, op0=ALU.max, op1=ALU.mult)
                    nc.sync.dma_start(out_d[nb:nb + span, :], outw[0:span, :])

            if debug:
                nc.sync.dma_start(dbg_h[:, :], h_full[:, :])
                nc.sync.dma_start(dbg_h1[:, :], h1loc[:, :])
                nc.sync.dma_start(dbg_xf[:, :], xl2full[:, :])

    nc.finalize()
    return nc


# ----------------------------------------------------------------------------
# entry point
# ----------------------------------------------------------------------------

def _install_ntff_hook():
    """Shim antenv.axon_hooks so trace=True can collect NTFF profiles."""
    import types
    try:
        from antenv.axon_hooks import get_axon_ntff_profile_hook  # noqa: F401
        return
    except ImportError:
        pass
    try:
        import antenv
        boot_dir = "/root/.axon_site/trn_agent_boot"
        so_path = "/opt/axon/libaxon_pjrt.so"
        if boot_dir not in sys.path:
            sys.path.insert(0, boot_dir)
        import trn_boot
        mod = types.ModuleType("antenv.axon_hooks")
        _state = {"hook": None}
        mod.set_axon_ntff_profile_hook = lambda h: _state.__setitem__("hook", h)
        mod.get_axon_ntff_profile_hook = lambda: _state["hook"]
        sys.modules["antenv.axon_hooks"] = mod
        antenv.axon_hooks = mod
        if os.path.exists(so_path):
            mod.set_axon_ntff_profile_hook(
                trn_boot._ntff_profile_via_ctypes(so_path))
    except Exception as exc:  # profiling is best-effort
        print("ntff hook install failed:", exc)


def run(inputs, trace=False, debug=False):
    if trace:
        _install_ntff_hook()
    n = int(inputs["x"].shape[0])
    e = int(inputs["edge_index"].shape[1])
    assert n % NCORES == 0
    npc = n // NCORES
    meta, in_maps, perms = _prep_host(inputs, n, e, npc)
    nc = _build_nc(meta, debug=debug)
    res = run_bass_kernel_spmd(nc, in_maps, list(range(NCORES)), trace=trace)
    full = np.empty((n, HC), np.float32)
    for c in range(NCORES):
        full[c * npc + perms[c]] = res.results[c]["out"]
    return full, res


def kernel(**inputs):
    full, _ = run(inputs, trace=False)
    return full


# revision 17
# speedup vs baseline: 1.0426x; 1.0426x over previous
"""GATv2FeatureExtractor Trainium2 kernel (8 NeuronCores, edge-parallel by dst).

v2 design
---------
Edges sorted by destination, sharded into 8 contiguous dst ranges (6250 nodes
per core); local node order is degree-balanced so fixed node-windows (55 nodes
for GAT layer 1, 119 for layer 2) carry near-equal edge counts.  The weighted
message scatter is a one-hot matmul into a PSUM window (no cross-core
reduction).  Two AllGathers: h after a *sharded* MLP encoder, and the layer-2
source table (h1 @ Wl2) between the GAT layers.

Key mechanics vs v1:
- Per-window batched indirect gather (one SWDGE descriptor-gen call per
  window instead of per 128-edge chunk: 994ns fixed overhead amortized).
- alpha via the relu decomposition  att . LReLU(s) = 0.2*(att.s) + 0.8*(att.relu(s)):
  the 0.2 linear term rides as 4 extra matmul columns; the relu term is ONE
  fused (0 max s)*att vector op + ONE segmented tensor_reduce per chunk.
- Payload scaling via a single broadcast tensor_tensor per chunk (ex
  broadcast over each head's 65-column block).
- One-hot rows built with a single broadcast is_equal per window.
- Per-window chunk counts (variable k) instead of a global max.
- Edge attrs DMA'd contiguously then strided into place on-chip (128 fat
  descriptors per window instead of 128*k tiny ones).

Compute in fp16 (f32 PSUM accumulation); the layer-2 exp bias column is
carried as an fp16 hi/lo pair.
"""

import os
import sys

import numpy as np

if os.path.isdir("/opt/trn_rl_repo") and "/opt/trn_rl_repo" not in sys.path:
    sys.path.insert(0, "/opt/trn_rl_repo")

import concourse.bacc as bacc
import concourse.bass as bass
import concourse.mybir as mybir
import concourse.tile as tile
from concourse.bass import IndirectOffsetOnAxis, broadcast_tensor_aps
from concourse.bass_utils import run_bass_kernel_spmd

F32 = mybir.dt.float32
BF16 = mybir.dt.float16  # 16-bit compute dtype (fp16: better mantissa, same speed)
I32 = mybir.dt.int32
AF = mybir.ActivationFunctionType
ALU = mybir.AluOpType

NCORES = 8
P = 128

F_IN, ED, HID, H, OUT = 32, 8, 64, 4, 256
HC = H * HID  # 256
XW = 260      # xl2 table width: 256 payload + tl (f32 in 2 slots) + ones + pad


def _bf(a):
    return np.asarray(a, np.float32).astype(np.float16)


def _ceil_div(a, b):
    return -(-a // b)


# ----------------------------------------------------------------------------
# host-side preprocessing
# ----------------------------------------------------------------------------

def _balanced_perms(deg, n, npc, span):
    """Per-core degree-balanced local permutation for `span`-node windows."""
    perms, invs = [], []
    for c in range(NCORES):
        d = deg[c * npc:(c + 1) * npc]
        order = np.argsort(-d, kind="stable")
        rows = np.arange(npc)
        seq = np.lexsort((rows // span, rows % span))
        perm = np.empty(npc, np.int64)
        perm[seq] = order
        inv = np.empty(npc, np.int64)
        inv[perm] = np.arange(npc)
        perms.append(perm)
        invs.append(inv)
    return perms, invs


def _pack_var(src_row, dstl, winid, ea_e, nw, kws, offs, sumk):
    """Pack one core's edges into variable-k chunk-major arrays [P, sumk]."""
    src_f = np.zeros((P, sumk), np.int32)
    dst_f = np.full((P, sumk), -1.0, np.float32)
    ea_f = np.zeros((P, sumk, ED), np.float32)
    order = np.argsort(winid, kind="stable")
    bounds = np.searchsorted(winid[order], np.arange(nw + 1))
    for w in range(nw):
        a, b = bounds[w], bounds[w + 1]
        m = b - a
        if m == 0:
            continue
        sel = order[a:b]
        jj = np.arange(m)
        lane, chunk = jj % P, offs[w] + jj // P
        src_f[lane, chunk] = src_row[sel]
        dst_f[lane, chunk] = dstl[sel]
        ea_f[lane, chunk] = ea_e[sel]
    return src_f, dst_f, _bf(ea_f.reshape(P, sumk * ED))


def _prep_host(inputs, n, e, npc, w1=55, w2=119):
    x = np.asarray(inputs["x"], np.float32)
    ei = np.asarray(inputs["edge_index"])
    ea = np.asarray(inputs["edge_attr"], np.float32)
    src = ei[0].astype(np.int64)
    dst = ei[1].astype(np.int64)

    deg = np.bincount(dst, minlength=n)
    perms, invs = _balanced_perms(deg, n, npc, w1)
    nw1 = _ceil_div(npc, w1)
    nw2 = _ceil_div(npc, w2)
    inv_all = np.concatenate(invs)
    grow = (src // npc) * npc + inv_all[src]  # global balanced row of src

    owner = dst // npc
    core_pack = []
    cnt1 = np.zeros((NCORES, nw1), np.int64)
    cnt2 = np.zeros((NCORES, nw2), np.int64)
    for c in range(NCORES):
        es = np.where(owner == c)[0]
        r = invs[c][dst[es] - c * npc]
        w1id = r // w1
        w2id = r // w2
        np.add.at(cnt1[c], w1id, 1)
        np.add.at(cnt2[c], w2id, 1)
        core_pack.append((es, r, w1id, w2id))
    kws1 = np.maximum(1, _ceil_div(cnt1.max(0), P)).astype(np.int64)
    kws2 = np.maximum(1, _ceil_div(cnt2.max(0), P)).astype(np.int64)
    offs1 = np.concatenate([[0], np.cumsum(kws1)])
    offs2 = np.concatenate([[0], np.cumsum(kws2)])
    sumk1, sumk2 = int(offs1[-1]), int(offs2[-1])

    packed1, packed2 = [], []
    for c in range(NCORES):
        es, r, w1id, w2id = core_pack[c]
        g = grow[es]
        packed1.append(_pack_var(g, r - w1id * w1, w1id, ea[es], nw1, kws1, offs1, sumk1))
        packed2.append(_pack_var(g, r - w2id * w2, w2id, ea[es], nw2, kws2, offs2, sumk2))

    # --- weights ---
    gi = lambda k: np.asarray(inputs[k], np.float32)
    W1, b1, W2, b2 = gi("W1"), gi("b1"), gi("W2"), gi("b2")
    Wl1, bl1, Wr1, br1 = gi("Wl1"), gi("bl1"), gi("Wr1"), gi("br1")
    We1, att1, bias1 = gi("We1"), gi("att1"), gi("bias1")
    Wl2, bl2, Wr2, br2 = gi("Wl2"), gi("bl2"), gi("Wr2"), gi("br2")
    We2, att2, bias2 = gi("We2"), gi("att2"), gi("bias2")

    consts = {}
    consts["ident"] = _bf(np.eye(P, dtype=np.float32))
    consts["ones1"] = _bf(np.ones((1, P), np.float32))
    consts["mlp1"] = _bf(np.concatenate([W1, b1[None, :]], 0))
    consts["mlp2"] = _bf(np.concatenate([W2, b2[None, :]], 0))

    amat1 = np.zeros((HC, H), np.float32)
    for h in range(H):
        amat1[h * HID:(h + 1) * HID, h] = att1[h]

    def _aug1(m):  # [_, 256] -> [_, 260] with 0.2*linear columns
        return np.concatenate([m, 0.2 * (m @ amat1)], 1)

    brow1 = (bl1 + br1)[None, :]
    rc1 = np.zeros((P, HC + H), np.float32)
    rc1[0:HID] = _aug1(Wl1)
    rc1[HID:HID + ED] = _aug1(We1)
    rc1[127] = _aug1(brow1)[0]
    consts["rc1"] = _bf(rc1)
    consts["wr1aug"] = _bf(_aug1(Wr1))
    # payload: per head [64 cols of Wl1 | ones col]
    rc2 = np.zeros((P, HC + H), np.float32)
    pb = bl1 + bias1
    for h in range(H):
        rc2[0:HID, 65 * h:65 * h + HID] = Wl1[:, HID * h:HID * (h + 1)]
        rc2[127, 65 * h:65 * h + HID] = pb[HID * h:HID * (h + 1)]
        rc2[127, 65 * h + HID] = 1.0
    consts["rc2"] = _bf(rc2)
    consts["attw1"] = _bf(np.tile(0.8 * att1.reshape(1, HC), (P, 1)))

    arow2 = att2.reshape(HC)
    a2m = arow2[:, None]

    def _aug2(m):  # [_, 256] -> [_, 257]
        return np.concatenate([m, 0.2 * (m @ a2m)], 1)

    brow2 = (br2 - bias2)[None, :]
    rc21 = np.zeros((P, HC + 1), np.float32)
    rc21[0:ED] = _aug2(We2)
    rc21[127] = _aug2(brow2)[0]
    consts["rc21"] = _bf(rc21)
    wr2 = _aug2(Wr2)
    consts["wr2a"], consts["wr2b"] = _bf(wr2[0:P]), _bf(wr2[P:2 * P])
    wl2 = _aug2(Wl2)
    consts["wl2a"], consts["wl2b"] = _bf(wl2[0:P]), _bf(wl2[P:2 * P])
    xb = (bl2 + bias2)[None, :]
    x2b = _aug2(xb)
    x2b[0, HC] -= 4.0  # exp overflow guard rides the tl column
    consts["xl2bias"] = _bf(x2b)
    consts["attw2"] = _bf(np.tile(0.8 * arow2[None, :], (P, 1)))

    nch0 = _ceil_div(npc, 512)
    npcpad = nch0 * 512

    in_maps = []
    for c in range(NCORES):
        lo = c * npc
        xt = np.zeros((F_IN + 1, npcpad), np.float16)
        xt[0:F_IN, :npc] = _bf(x[lo + perms[c]].T)
        xt[F_IN, :npc] = 1.0
        m = dict(consts)
        m["xt"] = xt
        m["src1"], m["dstl1"], m["ea1"] = packed1[c]
        m["src2"], m["dstl2"], m["ea2"] = packed2[c]
        in_maps.append(m)

    meta = dict(n=n, npc=npc, npcpad=npcpad, nch0=nch0,
                w1=w1, nw1=nw1, kws1=[int(v) for v in kws1], offs1=[int(v) for v in offs1],
                w2=w2, nw2=nw2, kws2=[int(v) for v in kws2], offs2=[int(v) for v in offs2],
                sumk1=sumk1, sumk2=sumk2,
                k1max=int(kws1.max()), k2max=int(kws2.max()))
    return meta, in_maps, perms


# ----------------------------------------------------------------------------
# device program
# ----------------------------------------------------------------------------

def _build_nc(meta, debug=False):
    n, npc, npcpad, nch0 = meta["n"], meta["npc"], meta["npcpad"], meta["nch0"]
    w1, nw1, kws1, offs1 = meta["w1"], meta["nw1"], meta["kws1"], meta["offs1"]
    w2, nw2, kws2, offs2 = meta["w2"], meta["nw2"], meta["kws2"], meta["offs2"]
    sumk1, sumk2 = meta["sumk1"], meta["sumk2"]
    k1max, k2max = meta["k1max"], meta["k2max"]

    nc = bacc.Bacc("TRN2", target_bir_lowering=False, num_devices=NCORES)

    def din(name, shape, dtype=BF16):
        return nc.dram_tensor(name, shape, dtype, kind="ExternalInput")

    ident_d = din("ident", [P, P])
    ones1_d = din("ones1", [1, P])
    mlp1_d = din("mlp1", [F_IN + 1, HID])
    mlp2_d = din("mlp2", [HID + 1, HID])
    rc1_d = din("rc1", [P, HC + H])
    rc2_d = din("rc2", [P, HC + H])
    wr1aug_d = din("wr1aug", [HID, HC + H])
    attw1_d = din("attw1", [P, HC])
    rc21_d = din("rc21", [P, HC + 1])
    wr2a_d = din("wr2a", [P, HC + 1]); wr2b_d = din("wr2b", [P, HC + 1])
    wl2a_d = din("wl2a", [P, HC + 1]); wl2b_d = din("wl2b", [P, HC + 1])
    xl2bias_d = din("xl2bias", [1, HC + 1])
    attw2_d = din("attw2", [P, HC])
    xt_d = din("xt", [F_IN + 1, npcpad])
    src1_d = din("src1", [P, sumk1], I32)
    dstl1_d = din("dstl1", [P, sumk1], F32)
    ea1_d = din("ea1", [P, sumk1 * ED])
    src2_d = din("src2", [P, sumk2], I32)
    dstl2_d = din("dstl2", [P, sumk2], F32)
    ea2_d = din("ea2", [P, sumk2 * ED])
    out_d = nc.dram_tensor("out", [npc, HC], F32, kind="ExternalOutput")
    if debug:
        dbg_h = nc.dram_tensor("dbg_h", [n, HID], BF16, kind="ExternalOutput")
        dbg_h1 = nc.dram_tensor("dbg_h1", [npc, HC], BF16, kind="ExternalOutput")
        dbg_xf = nc.dram_tensor("dbg_xf", [n, XW], BF16, kind="ExternalOutput")

    with tile.TileContext(nc) as tc:
        with (
            tc.tile_pool(name="dram", bufs=1, space="DRAM") as dram,
            tc.tile_pool(name="const", bufs=1) as cpool,
        ):
            hloc = dram.tile([npcpad, HID], BF16)
            h_full = dram.tile([n, HID], BF16)
            h1loc = dram.tile([npc, HC], BF16)
            xl2loc = dram.tile([npc, XW], BF16)
            xl2full = dram.tile([n, XW], BF16)

            def cload(name, shape, src_d, dt=BF16):
                t = cpool.tile(shape, dt, tag=name)
                nc.sync.dma_start(t[:], src_d[:, :])
                return t

            ident = cload("ident", [P, P], ident_d)
            ones1 = cload("ones1", [1, P], ones1_d)
            mlp1 = cload("mlp1", [F_IN + 1, HID], mlp1_d)
            mlp2 = cload("mlp2", [HID + 1, HID], mlp2_d)
            rc1 = cload("rc1", [P, HC + H], rc1_d)
            rc2 = cload("rc2", [P, HC + H], rc2_d)
            wr1aug = cload("wr1aug", [HID, HC + H], wr1aug_d)
            attw1 = cload("attw1", [P, HC], attw1_d)
            rc21 = cload("rc21", [P, HC + 1], rc21_d)
            wr2a = cload("wr2a", [P, HC + 1], wr2a_d)
            wr2b = cload("wr2b", [P, HC + 1], wr2b_d)
            wl2a = cload("wl2a", [P, HC + 1], wl2a_d)
            wl2b = cload("wl2b", [P, HC + 1], wl2b_d)
            xl2bias = cload("xl2bias", [1, HC + 1], xl2bias_d)
            attw2 = cload("attw2", [P, HC], attw2_d)
            iotaF = cpool.tile([P, w2], F32, tag="iotaF")
            nc.gpsimd.iota(iotaF[:], pattern=[[1, w2]], base=0,
                           channel_multiplier=0,
                           allow_small_or_imprecise_dtypes=True)
            neg4 = cpool.tile([P, 1], F32, tag="neg4")
            nc.vector.memset(neg4[:], -4.0)

            # ---------------- phase 0: sharded MLP encoder -> hloc -----------
            with (
                tc.tile_pool(name="mlp", bufs=2) as mpool,
                tc.tile_pool(name="mps", bufs=2, space="PSUM") as mps,
            ):
                for i in range(nch0):
                    sl = slice(i * 512, (i + 1) * 512)
                    rx = mpool.tile([F_IN + 1, 512], BF16, tag="rx")
                    nc.sync.dma_start(rx[:], xt_d[:, sl])
                    p1 = mps.tile([HID, 512], F32, tag="p1")
                    nc.tensor.matmul(p1[:], lhsT=mlp1[:], rhs=rx[:], start=True, stop=True)
                    ht = mpool.tile([HID + 1, 512], BF16, tag="ht")
                    nc.scalar.activation(ht[0:HID, :], p1[:], AF.Relu)
                    nc.vector.memset(ht[HID:HID + 1, :], 1.0)
                    p2 = mps.tile([HID, 512], F32, tag="p2")
                    nc.tensor.matmul(p2[:], lhsT=mlp2[:], rhs=ht[:], start=True, stop=True)
                    h2 = mpool.tile([HID, 512], BF16, tag="h2")
                    nc.scalar.activation(h2[:], p2[:], AF.Relu)
                    hrow = mpool.tile([P, 4, HID], BF16, tag="hrow")
                    for j in range(4):
                        pt = mps.tile([P, HID], BF16, tag="pt")
                        nc.tensor.transpose(pt[:], h2[:, j * P:(j + 1) * P],
                                            ident[0:HID, 0:HID])
                        nc.scalar.activation(hrow[:, j, :], pt[:], AF.Copy)
                    nc.sync.dma_start(
                        hloc[sl, :].rearrange("(j p) d -> p j d", p=P), hrow[:])

            # ---------------- allgather h ------------------------------------
            nc.gpsimd.collective_compute(
                "AllGather", ALU.bypass,
                replica_groups=[list(range(NCORES))],
                ins=[hloc[0:npc, :]], outs=[h_full[:]])

            # ---------------- phase 1: GAT layer 1 ---------------------------
            rc1w_a = cpool.tile([P, HC + H], BF16, tag="rc1w_a")
            rc1w_b = cpool.tile([P, HC + H], BF16, tag="rc1w_b")
            rc21w_a = cpool.tile([P, HC + 1], BF16, tag="rc21w_a")
            rc21w_b = cpool.tile([P, HC + 1], BF16, tag="rc21w_b")
            rc1ws = [rc1w_a, rc1w_b]
            rc21ws = [rc21w_a, rc21w_b]
            with (
                tc.tile_pool(name="w1p", bufs=2) as wpool,
                tc.tile_pool(name="e1p", bufs=3) as epool,
                tc.tile_pool(name="kp1", bufs=3) as kpool,
                tc.tile_pool(name="ps_s", bufs=2, space="PSUM") as ps_s,
                tc.tile_pool(name="ps_x", bufs=2, space="PSUM") as ps_x,
                tc.tile_pool(name="ps_t", bufs=2, space="PSUM") as ps_t,
                tc.tile_pool(name="ps_o", bufs=1, space="PSUM") as ps_o,
                tc.tile_pool(name="ps_p", bufs=1, space="PSUM") as ps_p,
            ):
                def prep1(w):
                    span = min(w1, npc - w * w1)
                    nb = w * w1
                    kw = kws1[w]
                    off = offs1[w]
                    hwin = wpool.tile([64, HID], BF16, tag="hwin")
                    nc.sync.dma_start(hwin[0:span, :], hloc[nb:nb + span, :])
                    srcw = epool.tile([P, k1max], I32, tag="srcw")
                    nc.sync.dma_start(srcw[:, 0:kw], src1_d[:, off:off + kw])
                    dstw = wpool.tile([P, k1max], F32, tag="dstw")
                    nc.sync.dma_start(dstw[:, 0:kw], dstl1_d[:, off:off + kw])
                    pre = epool.tile([P, k1max, P], BF16, tag="pre")
                    for c in range(kw):
                        nc.gpsimd.indirect_dma_start(
                            out=pre[:, c, 0:HID], out_offset=None,
                            in_=h_full[:, :],
                            in_offset=IndirectOffsetOnAxis(ap=srcw[:, c:c + 1], axis=0))
                    nc.sync.dma_start(
                        pre[:, 0:kw, HID:HID + ED],
                        ea1_d[:, off * ED:(off + kw) * ED].rearrange(
                            "p (k d) -> p k d", d=ED))
                    nc.vector.memset(pre[:, 0:kw, 127:P], 1.0)
                    i0, i1 = broadcast_tensor_aps(
                        iotaF[:, None, 0:w1], dstw[:, 0:kw, None])
                    nc.vector.tensor_tensor(
                        out=pre[:, 0:kw, 72:127], in0=i0, in1=i1, op=ALU.is_equal)
                    ptw = ps_t.tile([P, P], BF16, tag="ptp")
                    nc.tensor.transpose(ptw[0:HID, 0:span], hwin[0:span, :],
                                        ident[0:span, 0:span])
                    hwT = wpool.tile([HID, 64], BF16, tag="hwT")
                    nc.scalar.activation(hwT[:, 0:span], ptw[0:HID, 0:span], AF.Copy)
                    pxr = ps_p.tile([64, HC + H], F32, tag="pxp")
                    nc.tensor.matmul(pxr[0:span, :], lhsT=hwT[:, 0:span],
                                     rhs=wr1aug[:], start=True, stop=True)
                    rc1w = rc1ws[w % 2]
                    if w < 2:
                        nc.sync.dma_start(rc1w[:], rc1[:])
                    xrw = wpool.tile([64, HC + H], BF16, tag="xrw")
                    nc.scalar.activation(xrw[0:span, :], pxr[0:span, :], AF.Copy)
                    nc.sync.dma_start(rc1w[72:72 + span, :], xrw[0:span, :])
                    return pre, rc1w

                def chunks1(w, pre, rc1w):
                    span = min(w1, npc - w * w1)
                    nb = w * w1
                    kw = kws1[w]
                    stk = epool.tile([P, k1max, P], BF16, tag="stk")
                    for c in range(kw):
                        ptc = ps_t.tile([P, P], BF16, tag="ptp")
                        nc.tensor.transpose(ptc[:], pre[:, c, :], ident[:])
                        nc.scalar.activation(stk[:, c, :], ptc[:], AF.Copy)

                    pout = ps_o.tile([64, HC + H], F32, tag="pout")
                    for c in range(kw):
                        pss = ps_s.tile([P, HC + H], F32, tag="pss")
                        nc.tensor.matmul(pss[:], lhsT=stk[:, c, :], rhs=rc1w[:],
                                         start=True, stop=True)
                        psx = ps_x.tile([P, HC + H], F32, tag="psx")
                        nc.tensor.matmul(psx[:], lhsT=stk[:, c, :], rhs=rc2[:],
                                         start=True, stop=True)
                        m = kpool.tile([P, HC], BF16, tag="m")
                        nc.vector.scalar_tensor_tensor(
                            out=m[:], in0=pss[:, 0:HC], scalar=0.0,
                            in1=attw1[:], op0=ALU.max, op1=ALU.mult)
                        alph = kpool.tile([P, H], F32, tag="alph")
                        nc.vector.tensor_reduce(
                            out=alph[:, :, None],
                            in_=m[:, :].rearrange("p (h x) -> p h x", x=HID),
                            axis=mybir.AxisListType.X, op=ALU.add)
                        alph2 = kpool.tile([P, H], F32, tag="alph2")
                        nc.vector.tensor_tensor(
                            out=alph2[:], in0=alph[:], in1=pss[:, HC:HC + H],
                            op=ALU.add)
                        ex = kpool.tile([P, H], F32, tag="ex")
                        nc.scalar.activation(ex[:], alph2[:], AF.Exp, bias=neg4[:])
                        pay = kpool.tile([P, HC + H], BF16, tag="pay")
                        x0, x1 = broadcast_tensor_aps(
                            psx[:, :].rearrange("p (h x) -> p h x", x=65),
                            ex[:, :, None])
                        nc.vector.tensor_tensor(
                            out=pay[:, :].rearrange("p (h x) -> p h x", x=65),
                            in0=x0, in1=x1, op=ALU.mult)
                        nc.tensor.matmul(pout[0:span, :],
                                         lhsT=pre[:, c, 72:72 + span],
                                         rhs=pay[:], start=(c == 0),
                                         stop=(c == kw - 1))

                    # normalize + relu -> h1 window; prep xl2 rows
                    pog = pout[0:span, :].rearrange("p (h x) -> p h x", x=65)
                    deng = wpool.tile([64, H], F32, tag="deng")
                    nc.vector.tensor_scalar(
                        out=deng[0:span, :, None], in0=pog[:, :, 64:65],
                        scalar1=1e-30, scalar2=None, op0=ALU.max)
                    rden = wpool.tile([64, H], F32, tag="rden")
                    nc.vector.reciprocal(rden[0:span, :], deng[0:span, :])
                    h1w = wpool.tile([64, HC], BF16, tag="h1w")
                    if span < 64:
                        nc.vector.memset(h1w[:], 0.0)
                    r0, r1 = broadcast_tensor_aps(
                        pog[:, :, 0:64], rden[0:span, :, None])
                    nc.vector.scalar_tensor_tensor(
                        out=h1w[0:span, :].rearrange("p (h x) -> p h x", x=HID),
                        in0=r0, scalar=0.0, in1=r1, op0=ALU.max, op1=ALU.mult)
                    nc.sync.dma_start(h1loc[nb:nb + span, :], h1w[0:span, :])

                    h1T = wpool.tile([P, 2, 64], BF16, tag="h1T")
                    for j in range(2):
                        ptj = ps_t.tile([P, P], BF16, tag="ptp")
                        nc.tensor.transpose(ptj[:, 0:span],
                                            h1w[0:span, j * P:(j + 1) * P],
                                            ident[0:span, 0:span])
                        nc.scalar.activation(h1T[:, j, 0:span], ptj[:, 0:span],
                                             AF.Copy)
                    pxl2 = ps_p.tile([64, HC + H], F32, tag="pxp")
                    nc.tensor.matmul(pxl2[0:span, 0:HC + 1], lhsT=h1T[:, 0, 0:span],
                                     rhs=wl2a[:], start=True, stop=False)
                    nc.tensor.matmul(pxl2[0:span, 0:HC + 1], lhsT=h1T[:, 1, 0:span],
                                     rhs=wl2b[:], start=False, stop=False)
                    nc.tensor.matmul(pxl2[0:span, 0:HC + 1], lhsT=ones1[:, 0:span],
                                     rhs=xl2bias[:], start=False, stop=True)
                    xl2w = wpool.tile([64, XW], BF16, tag="xl2w")
                    nc.scalar.activation(xl2w[0:span, 0:HC], pxl2[0:span, 0:HC],
                                         AF.Copy)
                    nc.vector.tensor_copy(
                        xl2w[0:span, HC:HC + 2].bitcast(F32),
                        pxl2[0:span, HC:HC + 1])
                    nc.vector.memset(xl2w[0:span, HC + 2:HC + 3], 1.0)
                    nc.vector.memset(xl2w[0:span, HC + 3:XW], 0.0)
                    nc.sync.dma_start(xl2loc[nb:nb + span, :], xl2w[0:span, :])

                state1 = prep1(0)
                for w in range(nw1):
                    nxt = prep1(w + 1) if w + 1 < nw1 else None
                    chunks1(w, *state1)
                    state1 = nxt

            # ---------------- phase 2: allgather xl2 table -------------------
            nc.gpsimd.collective_compute(
                "AllGather", ALU.bypass,
                replica_groups=[list(range(NCORES))],
                ins=[xl2loc[:]], outs=[xl2full[:]])

            # ---------------- phase 3: GAT layer 2 ---------------------------
            with (
                tc.tile_pool(name="w2p", bufs=2) as wpool,
                tc.tile_pool(name="e2p", bufs=3) as epool,
                tc.tile_pool(name="kp2", bufs=3) as kpool,
                tc.tile_pool(name="ps2_s", bufs=2, space="PSUM") as ps2_s,
                tc.tile_pool(name="ps2_t", bufs=2, space="PSUM") as ps2_t,
                tc.tile_pool(name="ps2_o", bufs=1, space="PSUM") as ps2_o,
                tc.tile_pool(name="ps2_p", bufs=1, space="PSUM") as ps2_p,
            ):
                def prep2(w):
                    span = min(w2, npc - w * w2)
                    nb = w * w2
                    kw = kws2[w]
                    off = offs2[w]
                    h1r = wpool.tile([P, HC], BF16, tag="h1r")
                    if span < P:
                        nc.vector.memset(h1r[:], 0.0)
                    nc.sync.dma_start(h1r[0:span, :], h1loc[nb:nb + span, :])
                    srcw2 = epool.tile([P, k2max], I32, tag="srcw2")
                    nc.sync.dma_start(srcw2[:, 0:kw], src2_d[:, off:off + kw])
                    xg = epool.tile([P, k2max, XW], BF16, tag="xg")
                    for c in range(kw):
                        nc.gpsimd.indirect_dma_start(
                            out=xg[:, c, :], out_offset=None,
                            in_=xl2full[:, :],
                            in_offset=IndirectOffsetOnAxis(ap=srcw2[:, c:c + 1], axis=0))
                    dstw2 = wpool.tile([P, k2max], F32, tag="dstw2")
                    nc.sync.dma_start(dstw2[:, 0:kw], dstl2_d[:, off:off + kw])
                    pre2 = epool.tile([P, k2max, P], BF16, tag="pre2")
                    nc.sync.dma_start(
                        pre2[:, 0:kw, 0:ED],
                        ea2_d[:, off * ED:(off + kw) * ED].rearrange(
                            "p (k d) -> p k d", d=ED))
                    nc.vector.memset(pre2[:, 0:kw, 127:P], 1.0)
                    i0, i1 = broadcast_tensor_aps(
                        iotaF[:, None, 0:w2], dstw2[:, 0:kw, None])
                    nc.vector.tensor_tensor(
                        out=pre2[:, 0:kw, ED:ED + w2], in0=i0, in1=i1,
                        op=ALU.is_equal)
                    h1rT = wpool.tile([P, 2, P], BF16, tag="h1rT")
                    for j in range(2):
                        ptj = ps2_t.tile([P, P], BF16, tag="ptp2")
                        nc.tensor.transpose(ptj[:, 0:span],
                                            h1r[0:span, j * P:(j + 1) * P],
                                            ident[0:span, 0:span])
                        nc.scalar.activation(h1rT[:, j, 0:span], ptj[:, 0:span],
                                             AF.Copy)
                    pxr2 = ps2_p.tile([P, HC + 1], F32, tag="pxr2")
                    nc.tensor.matmul(pxr2[0:span, :], lhsT=h1rT[:, 0, 0:span],
                                     rhs=wr2a[:], start=True, stop=False)
                    nc.tensor.matmul(pxr2[0:span, :], lhsT=h1rT[:, 1, 0:span],
                                     rhs=wr2b[:], start=False, stop=True)
                    rc21w = rc21ws[w % 2]
                    if w < 2:
                        nc.sync.dma_start(rc21w[:], rc21[:])
                    xrw2 = wpool.tile([P, HC + 1], BF16, tag="xrw2")
                    nc.scalar.activation(xrw2[0:span, :], pxr2[0:span, :], AF.Copy)
                    nc.sync.dma_start(rc21w[ED:ED + span, :], xrw2[0:span, :])
                    return pre2, xg, rc21w

                def chunks2(w, pre2, xg, rc21w):
                    span = min(w2, npc - w * w2)
                    nb = w * w2
                    kw = kws2[w]
                    ng = _ceil_div(kw, 2)
                    stk2 = epool.tile([P, k2max, P], BF16, tag="stk2")
                    for c in range(kw):
                        ptc = ps2_t.tile([P, P], BF16, tag="ptp2")
                        nc.tensor.transpose(ptc[:], pre2[:, c, :], ident[:])
                        nc.scalar.activation(stk2[:, c, :], ptc[:], AF.Copy)

                    pout2 = ps2_o.tile([P, HC + 3], F32, tag="pout2")
                    for g in range(ng):
                        c0 = 2 * g
                        gw = min(2, kw - c0)
                        pss2 = ps2_s.tile([P, 2, 512], F32, tag="pss2")
                        for j in range(gw):
                            nc.tensor.matmul(
                                pss2[:, j, 0:HC + 1], lhsT=stk2[:, c0 + j, :],
                                rhs=rc21w[:], start=True, stop=True)
                        s2 = kpool.tile([P, 2, HC], BF16, tag="s2")
                        nc.vector.tensor_tensor(
                            out=s2[:, 0:gw, :], in0=pss2[:, 0:gw, 0:HC],
                            in1=xg[:, c0:c0 + gw, 0:HC], op=ALU.add)
                        m2 = kpool.tile([P, 2, HC], BF16, tag="m2")
                        a0, a1 = broadcast_tensor_aps(
                            s2[:, 0:gw, :], attw2[:, None, :])
                        nc.vector.scalar_tensor_tensor(
                            out=m2[:, 0:gw, :], in0=a0, scalar=0.0,
                            in1=a1, op0=ALU.max, op1=ALU.mult)
                        al2 = kpool.tile([P, 2], F32, tag="al2")
                        nc.vector.tensor_reduce(
                            out=al2[:, 0:gw, None], in_=m2[:, 0:gw, :],
                            axis=mybir.AxisListType.X, op=ALU.add)
                        al2b = kpool.tile([P, 2], F32, tag="al2b")
                        nc.vector.tensor_tensor(
                            out=al2b[:, 0:gw, None], in0=al2[:, 0:gw, None],
                            in1=pss2[:, 0:gw, HC:HC + 1], op=ALU.add)
                        al2c = kpool.tile([P, 2], F32, tag="al2c")
                        nc.vector.tensor_tensor(
                            out=al2c[:, 0:gw, None], in0=al2b[:, 0:gw, None],
                            in1=xg[:, c0:c0 + gw, HC:HC + 2].bitcast(F32),
                            op=ALU.add)
                        ex2 = kpool.tile([P, 2], F32, tag="ex2")
                        nc.scalar.activation(ex2[:, 0:gw], al2c[:, 0:gw], AF.Exp)
                        pay2 = kpool.tile([P, 2, HC + 3], BF16, tag="pay2")
                        for j in range(gw):
                            nc.vector.tensor_scalar(
                                out=pay2[:, j, :], in0=xg[:, c0 + j, 0:HC + 3],
                                scalar1=ex2[:, j:j + 1], scalar2=None,
                                op0=ALU.mult)
                        for j in range(gw):
                            c = c0 + j
                            nc.tensor.matmul(
                                pout2[0:span, :], lhsT=pre2[:, c, ED:ED + span],
                                rhs=pay2[:, j, :], start=(c == 0),
                                stop=(c == kw - 1))

                    deng2 = wpool.tile([P, 1], F32, tag="deng2")
                    nc.vector.tensor_scalar(
                        out=deng2[0:span, :], in0=pout2[0:span, HC + 2:HC + 3],
                        scalar1=1e-30, scalar2=None, op0=ALU.max)
                    rden2 = wpool.tile([P, 1], F32, tag="rden2")
                    nc.vector.reciprocal(rden2[0:span, :], deng2[0:span, :])
                    outw = wpool.tile([P, HC], F32, tag="outw")
                    b0, b1 = broadcast_tensor_aps(
                        pout2[0:span, 0:HC], rden2[0:span, :])
                    nc.vector.scalar_tensor_tensor(
                        out=outw[0:span, :], in0=b0, scalar=0.0,
                        in1=b1, op0=ALU.max, op1=ALU.mult)
                    nc.sync.dma_start(out_d[nb:nb + span, :], outw[0:span, :])

                state2 = prep2(0)
                for w in range(nw2):
                    nxt = prep2(w + 1) if w + 1 < nw2 else None
                    chunks2(w, *state2)
                    state2 = nxt

            if debug:
                nc.sync.dma_start(dbg_h[:, :], h_full[:, :])
                nc.sync.dma_start(dbg_h1[:, :], h1loc[:, :])
                nc.sync.dma_start(dbg_xf[:, :], xl2full[:, :])

    nc.finalize()
    return nc


# ----------------------------------------------------------------------------
# entry point
# ----------------------------------------------------------------------------

def _install_ntff_hook():
    """Shim antenv.axon_hooks so trace=True can collect NTFF profiles."""
    import types
    try:
        from antenv.axon_hooks import get_axon_ntff_profile_hook  # noqa: F401
        return
    except ImportError:
        pass
    try:
        import antenv
        boot_dir = "/root/.axon_site/trn_agent_boot"
        so_path = "/opt/axon/libaxon_pjrt.so"
        if boot_dir not in sys.path:
            sys.path.insert(0, boot_dir)
        import trn_boot
        mod = types.ModuleType("antenv.axon_hooks")
        _state = {"hook": None}
        mod.set_axon_ntff_profile_hook = lambda h: _state.__setitem__("hook", h)
        mod.get_axon_ntff_profile_hook = lambda: _state["hook"]
        sys.modules["antenv.axon_hooks"] = mod
        antenv.axon_hooks = mod
        if os.path.exists(so_path):
            mod.set_axon_ntff_profile_hook(
                trn_boot._ntff_profile_via_ctypes(so_path))
    except Exception as exc:  # profiling is best-effort
        print("ntff hook install failed:", exc)


def run(inputs, trace=False, debug=False):
    if trace:
        _install_ntff_hook()
    n = int(inputs["x"].shape[0])
    e = int(inputs["edge_index"].shape[1])
    assert n % NCORES == 0
    npc = n // NCORES
    meta, in_maps, perms = _prep_host(inputs, n, e, npc)
    nc = _build_nc(meta, debug=debug)
    res = run_bass_kernel_spmd(nc, in_maps, list(range(NCORES)), trace=trace)
    full = np.empty((n, HC), np.float32)
    for c in range(NCORES):
        full[c * npc + perms[c]] = res.results[c]["out"]
    return full, res


def kernel(**inputs):
    full, _ = run(inputs, trace=False)
    return full


# revision 19
# speedup vs baseline: 1.1199x; 1.0741x over previous
"""GATv2FeatureExtractor Trainium2 kernel (8 NeuronCores, edge-parallel by dst).

v2 design
---------
Edges sorted by destination, sharded into 8 contiguous dst ranges (6250 nodes
per core); local node order is degree-balanced so fixed node-windows (55 nodes
for GAT layer 1, 119 for layer 2) carry near-equal edge counts.  The weighted
message scatter is a one-hot matmul into a PSUM window (no cross-core
reduction).  Two AllGathers: h after a *sharded* MLP encoder, and the layer-2
source table (h1 @ Wl2) between the GAT layers.

Key mechanics vs v1:
- Per-window batched indirect gather (one SWDGE descriptor-gen call per
  window instead of per 128-edge chunk: 994ns fixed overhead amortized).
- alpha via the relu decomposition  att . LReLU(s) = 0.2*(att.s) + 0.8*(att.relu(s)):
  the 0.2 linear term rides as 4 extra matmul columns; the relu term is ONE
  fused (0 max s)*att vector op + ONE segmented tensor_reduce per chunk.
- Payload scaling via a single broadcast tensor_tensor per chunk (ex
  broadcast over each head's 65-column block).
- One-hot rows built with a single broadcast is_equal per window.
- Per-window chunk counts (variable k) instead of a global max.
- Edge attrs DMA'd contiguously then strided into place on-chip (128 fat
  descriptors per window instead of 128*k tiny ones).

Compute in fp16 (f32 PSUM accumulation); the layer-2 exp bias column is
carried as an fp16 hi/lo pair.
"""

import os
import sys

import numpy as np

if os.path.isdir("/opt/trn_rl_repo") and "/opt/trn_rl_repo" not in sys.path:
    sys.path.insert(0, "/opt/trn_rl_repo")

import concourse.bacc as bacc
import concourse.bass as bass
import concourse.mybir as mybir
import concourse.tile as tile
from concourse.bass import IndirectOffsetOnAxis, broadcast_tensor_aps
from concourse.bass_utils import run_bass_kernel_spmd

F32 = mybir.dt.float32
BF16 = mybir.dt.float16  # 16-bit compute dtype (fp16: better mantissa, same speed)
I32 = mybir.dt.int32
AF = mybir.ActivationFunctionType
ALU = mybir.AluOpType

NCORES = 8
P = 128

F_IN, ED, HID, H, OUT = 32, 8, 64, 4, 256
HC = H * HID  # 256
XW = 260      # xl2 table width: 256 payload + tl (f32 in 2 slots) + ones + pad


def _bf(a):
    return np.asarray(a, np.float32).astype(np.float16)


def _ceil_div(a, b):
    return -(-a // b)


# ----------------------------------------------------------------------------
# host-side preprocessing
# ----------------------------------------------------------------------------

def _balanced_perms(deg, n, npc, span):
    """Per-core degree-balanced local permutation for `span`-node windows."""
    perms, invs = [], []
    for c in range(NCORES):
        d = deg[c * npc:(c + 1) * npc]
        order = np.argsort(-d, kind="stable")
        rows = np.arange(npc)
        seq = np.lexsort((rows // span, rows % span))
        perm = np.empty(npc, np.int64)
        perm[seq] = order
        inv = np.empty(npc, np.int64)
        inv[perm] = np.arange(npc)
        perms.append(perm)
        invs.append(inv)
    return perms, invs


def _pack_var(src_row, dstl, winid, ea_e, nw, kws, offs, sumk):
    """Pack one core's edges into variable-k chunk-major arrays [P, sumk]."""
    src_f = np.zeros((P, sumk), np.int32)
    dst_f = np.full((P, sumk), -1.0, np.float32)
    ea_f = np.zeros((P, sumk, ED), np.float32)
    order = np.argsort(winid, kind="stable")
    bounds = np.searchsorted(winid[order], np.arange(nw + 1))
    for w in range(nw):
        a, b = bounds[w], bounds[w + 1]
        m = b - a
        if m == 0:
            continue
        sel = order[a:b]
        jj = np.arange(m)
        lane, chunk = jj % P, offs[w] + jj // P
        src_f[lane, chunk] = src_row[sel]
        dst_f[lane, chunk] = dstl[sel]
        ea_f[lane, chunk] = ea_e[sel]
    return src_f, dst_f, _bf(ea_f.reshape(P, sumk * ED))


def _prep_host(inputs, n, e, npc, w1=55, w2=119):
    x = np.asarray(inputs["x"], np.float32)
    ei = np.asarray(inputs["edge_index"])
    ea = np.asarray(inputs["edge_attr"], np.float32)
    src = ei[0].astype(np.int64)
    dst = ei[1].astype(np.int64)

    deg = np.bincount(dst, minlength=n)
    perms, invs = _balanced_perms(deg, n, npc, w1)
    nw1 = _ceil_div(npc, w1)
    nw2 = _ceil_div(npc, w2)
    inv_all = np.concatenate(invs)
    grow = (src // npc) * npc + inv_all[src]  # global balanced row of src

    owner = dst // npc
    core_pack = []
    cnt1 = np.zeros((NCORES, nw1), np.int64)
    cnt2 = np.zeros((NCORES, nw2), np.int64)
    for c in range(NCORES):
        es = np.where(owner == c)[0]
        r = invs[c][dst[es] - c * npc]
        w1id = r // w1
        w2id = r // w2
        np.add.at(cnt1[c], w1id, 1)
        np.add.at(cnt2[c], w2id, 1)
        core_pack.append((es, r, w1id, w2id))
    kws1 = np.maximum(1, _ceil_div(cnt1.max(0), P)).astype(np.int64)
    kws2 = np.maximum(1, _ceil_div(cnt2.max(0), P)).astype(np.int64)
    offs1 = np.concatenate([[0], np.cumsum(kws1)])
    offs2 = np.concatenate([[0], np.cumsum(kws2)])
    sumk1, sumk2 = int(offs1[-1]), int(offs2[-1])

    packed1, packed2 = [], []
    for c in range(NCORES):
        es, r, w1id, w2id = core_pack[c]
        g = grow[es]
        packed1.append(_pack_var(g, r - w1id * w1, w1id, ea[es], nw1, kws1, offs1, sumk1))
        packed2.append(_pack_var(g, r - w2id * w2, w2id, ea[es], nw2, kws2, offs2, sumk2))

    # --- weights ---
    gi = lambda k: np.asarray(inputs[k], np.float32)
    W1, b1, W2, b2 = gi("W1"), gi("b1"), gi("W2"), gi("b2")
    Wl1, bl1, Wr1, br1 = gi("Wl1"), gi("bl1"), gi("Wr1"), gi("br1")
    We1, att1, bias1 = gi("We1"), gi("att1"), gi("bias1")
    Wl2, bl2, Wr2, br2 = gi("Wl2"), gi("bl2"), gi("Wr2"), gi("br2")
    We2, att2, bias2 = gi("We2"), gi("att2"), gi("bias2")

    consts = {}
    consts["ident"] = _bf(np.eye(P, dtype=np.float32))
    consts["ones1"] = _bf(np.ones((1, P), np.float32))
    consts["mlp1"] = _bf(np.concatenate([W1, b1[None, :]], 0))
    consts["mlp2"] = _bf(np.concatenate([W2, b2[None, :]], 0))

    amat1 = np.zeros((HC, H), np.float32)
    for h in range(H):
        amat1[h * HID:(h + 1) * HID, h] = att1[h]

    def _aug1(m):  # [_, 256] -> [_, 260] with 0.2*linear columns
        return np.concatenate([m, 0.2 * (m @ amat1)], 1)

    brow1 = (bl1 + br1)[None, :]
    rc1 = np.zeros((P, HC + H), np.float32)
    rc1[0:HID] = _aug1(Wl1)
    rc1[HID:HID + ED] = _aug1(We1)
    rc1[127] = _aug1(brow1)[0]
    consts["rc1"] = _bf(rc1)
    consts["wr1aug"] = _bf(_aug1(Wr1))
    # payload: per head [64 cols of Wl1 | ones col]
    rc2 = np.zeros((P, HC + H), np.float32)
    pb = bl1 + bias1
    for h in range(H):
        rc2[0:HID, 65 * h:65 * h + HID] = Wl1[:, HID * h:HID * (h + 1)]
        rc2[127, 65 * h:65 * h + HID] = pb[HID * h:HID * (h + 1)]
        rc2[127, 65 * h + HID] = 1.0
    consts["rc2"] = _bf(rc2)
    consts["attw1"] = _bf(np.tile(0.8 * att1.reshape(1, HC), (P, 1)))

    arow2 = att2.reshape(HC)
    a2m = arow2[:, None]

    def _aug2(m):  # [_, 256] -> [_, 257]
        return np.concatenate([m, 0.2 * (m @ a2m)], 1)

    brow2 = (br2 - bias2)[None, :]
    rc21 = np.zeros((P, HC + 1), np.float32)
    rc21[0:ED] = _aug2(We2)
    rc21[127] = _aug2(brow2)[0]
    consts["rc21"] = _bf(rc21)
    wr2 = _aug2(Wr2)
    consts["wr2a"], consts["wr2b"] = _bf(wr2[0:P]), _bf(wr2[P:2 * P])
    wl2 = _aug2(Wl2)
    consts["wl2a"], consts["wl2b"] = _bf(wl2[0:P]), _bf(wl2[P:2 * P])
    xb = (bl2 + bias2)[None, :]
    x2b = _aug2(xb)
    x2b[0, HC] -= 4.0  # exp overflow guard rides the tl column
    consts["xl2bias"] = _bf(x2b)
    consts["attw2"] = _bf(np.tile(0.8 * arow2[None, :], (P, 1)))

    nch0 = _ceil_div(npc, 512)
    npcpad = nch0 * 512

    in_maps = []
    for c in range(NCORES):
        lo = c * npc
        xt = np.zeros((F_IN + 1, npcpad), np.float16)
        xt[0:F_IN, :npc] = _bf(x[lo + perms[c]].T)
        xt[F_IN, :npc] = 1.0
        m = dict(consts)
        m["xt"] = xt
        m["src1"], m["dstl1"], m["ea1"] = packed1[c]
        m["src2"], m["dstl2"], m["ea2"] = packed2[c]
        in_maps.append(m)

    meta = dict(n=n, npc=npc, npcpad=npcpad, nch0=nch0,
                w1=w1, nw1=nw1, kws1=[int(v) for v in kws1], offs1=[int(v) for v in offs1],
                w2=w2, nw2=nw2, kws2=[int(v) for v in kws2], offs2=[int(v) for v in offs2],
                sumk1=sumk1, sumk2=sumk2,
                k1max=int(kws1.max()), k2max=int(kws2.max()))
    return meta, in_maps, perms


# ----------------------------------------------------------------------------
# device program
# ----------------------------------------------------------------------------

def _build_nc(meta, debug=False):
    n, npc, npcpad, nch0 = meta["n"], meta["npc"], meta["npcpad"], meta["nch0"]
    w1, nw1, kws1, offs1 = meta["w1"], meta["nw1"], meta["kws1"], meta["offs1"]
    w2, nw2, kws2, offs2 = meta["w2"], meta["nw2"], meta["kws2"], meta["offs2"]
    sumk1, sumk2 = meta["sumk1"], meta["sumk2"]
    k1max, k2max = meta["k1max"], meta["k2max"]

    nc = bacc.Bacc("TRN2", target_bir_lowering=False, num_devices=NCORES)

    def din(name, shape, dtype=BF16):
        return nc.dram_tensor(name, shape, dtype, kind="ExternalInput")

    ident_d = din("ident", [P, P])
    ones1_d = din("ones1", [1, P])
    mlp1_d = din("mlp1", [F_IN + 1, HID])
    mlp2_d = din("mlp2", [HID + 1, HID])
    rc1_d = din("rc1", [P, HC + H])
    rc2_d = din("rc2", [P, HC + H])
    wr1aug_d = din("wr1aug", [HID, HC + H])
    attw1_d = din("attw1", [P, HC])
    rc21_d = din("rc21", [P, HC + 1])
    wr2a_d = din("wr2a", [P, HC + 1]); wr2b_d = din("wr2b", [P, HC + 1])
    wl2a_d = din("wl2a", [P, HC + 1]); wl2b_d = din("wl2b", [P, HC + 1])
    xl2bias_d = din("xl2bias", [1, HC + 1])
    attw2_d = din("attw2", [P, HC])
    xt_d = din("xt", [F_IN + 1, npcpad])
    src1_d = din("src1", [P, sumk1], I32)
    dstl1_d = din("dstl1", [P, sumk1], F32)
    ea1_d = din("ea1", [P, sumk1 * ED])
    src2_d = din("src2", [P, sumk2], I32)
    dstl2_d = din("dstl2", [P, sumk2], F32)
    ea2_d = din("ea2", [P, sumk2 * ED])
    out_d = nc.dram_tensor("out", [npc, HC], F32, kind="ExternalOutput")
    if debug:
        dbg_h = nc.dram_tensor("dbg_h", [n, HID], BF16, kind="ExternalOutput")
        dbg_h1 = nc.dram_tensor("dbg_h1", [npc, HC], BF16, kind="ExternalOutput")
        dbg_xf = nc.dram_tensor("dbg_xf", [n, XW], BF16, kind="ExternalOutput")

    with tile.TileContext(nc) as tc:
        with (
            tc.tile_pool(name="dram", bufs=1, space="DRAM") as dram,
            tc.tile_pool(name="const", bufs=1) as cpool,
        ):
            hloc = dram.tile([npcpad, HID], BF16)
            h_full = dram.tile([n, HID], BF16)
            h1loc = dram.tile([npc, HC], BF16)
            xl2loc = dram.tile([npc, XW], BF16)
            xl2full = dram.tile([n, XW], BF16)

            def cload(name, shape, src_d, dt=BF16):
                t = cpool.tile(shape, dt, tag=name)
                nc.sync.dma_start(t[:], src_d[:, :])
                return t

            ident = cload("ident", [P, P], ident_d)
            ones1 = cload("ones1", [1, P], ones1_d)
            mlp1 = cload("mlp1", [F_IN + 1, HID], mlp1_d)
            mlp2 = cload("mlp2", [HID + 1, HID], mlp2_d)
            rc1 = cload("rc1", [P, HC + H], rc1_d)
            rc2 = cload("rc2", [P, HC + H], rc2_d)
            wr1aug = cload("wr1aug", [HID, HC + H], wr1aug_d)
            attw1 = cload("attw1", [P, HC], attw1_d)
            rc21 = cload("rc21", [P, HC + 1], rc21_d)
            wr2a = cload("wr2a", [P, HC + 1], wr2a_d)
            wr2b = cload("wr2b", [P, HC + 1], wr2b_d)
            wl2a = cload("wl2a", [P, HC + 1], wl2a_d)
            wl2b = cload("wl2b", [P, HC + 1], wl2b_d)
            xl2bias = cload("xl2bias", [1, HC + 1], xl2bias_d)
            attw2 = cload("attw2", [P, HC], attw2_d)
            iotaF = cpool.tile([P, w2], F32, tag="iotaF")
            nc.gpsimd.iota(iotaF[:], pattern=[[1, w2]], base=0,
                           channel_multiplier=0,
                           allow_small_or_imprecise_dtypes=True)
            neg4 = cpool.tile([P, 1], F32, tag="neg4")
            nc.vector.memset(neg4[:], -4.0)

            # ---------------- phase 0: sharded MLP encoder -> hloc -----------
            with (
                tc.tile_pool(name="mlp", bufs=2) as mpool,
                tc.tile_pool(name="mps", bufs=2, space="PSUM") as mps,
            ):
                for i in range(nch0):
                    sl = slice(i * 512, (i + 1) * 512)
                    rx = mpool.tile([F_IN + 1, 512], BF16, tag="rx")
                    nc.sync.dma_start(rx[:], xt_d[:, sl])
                    p1 = mps.tile([HID, 512], F32, tag="p1")
                    nc.tensor.matmul(p1[:], lhsT=mlp1[:], rhs=rx[:], start=True, stop=True)
                    ht = mpool.tile([HID + 1, 512], BF16, tag="ht")
                    nc.scalar.activation(ht[0:HID, :], p1[:], AF.Relu)
                    nc.vector.memset(ht[HID:HID + 1, :], 1.0)
                    p2 = mps.tile([HID, 512], F32, tag="p2")
                    nc.tensor.matmul(p2[:], lhsT=mlp2[:], rhs=ht[:], start=True, stop=True)
                    h2 = mpool.tile([HID, 512], BF16, tag="h2")
                    nc.scalar.activation(h2[:], p2[:], AF.Relu)
                    hrow = mpool.tile([P, 4, HID], BF16, tag="hrow")
                    for j in range(4):
                        pt = mps.tile([P, HID], BF16, tag="pt")
                        nc.tensor.transpose(pt[:], h2[:, j * P:(j + 1) * P],
                                            ident[0:HID, 0:HID])
                        nc.scalar.activation(hrow[:, j, :], pt[:], AF.Copy)
                    nc.sync.dma_start(
                        hloc[sl, :].rearrange("(j p) d -> p j d", p=P), hrow[:])

            # ---------------- allgather h ------------------------------------
            nc.gpsimd.collective_compute(
                "AllGather", ALU.bypass,
                replica_groups=[list(range(NCORES))],
                ins=[hloc[0:npc, :]], outs=[h_full[:]])

            # ---------------- phase 1: GAT layer 1 ---------------------------
            rc1w_a = cpool.tile([P, HC + H], BF16, tag="rc1w_a")
            rc1w_b = cpool.tile([P, HC + H], BF16, tag="rc1w_b")
            rc21w_a = cpool.tile([P, HC + 1], BF16, tag="rc21w_a")
            rc21w_b = cpool.tile([P, HC + 1], BF16, tag="rc21w_b")
            rc1ws = [rc1w_a, rc1w_b]
            rc21ws = [rc21w_a, rc21w_b]
            with (
                tc.tile_pool(name="w1p", bufs=2) as wpool,
                tc.tile_pool(name="e1p", bufs=3) as epool,
                tc.tile_pool(name="kp1", bufs=3) as kpool,
                tc.tile_pool(name="ps_s", bufs=2, space="PSUM") as ps_s,
                tc.tile_pool(name="ps_x", bufs=2, space="PSUM") as ps_x,
                tc.tile_pool(name="ps_t", bufs=2, space="PSUM") as ps_t,
                tc.tile_pool(name="ps_o", bufs=1, space="PSUM") as ps_o,
                tc.tile_pool(name="ps_p", bufs=1, space="PSUM") as ps_p,
            ):
                def prep1(w):
                    span = min(w1, npc - w * w1)
                    nb = w * w1
                    kw = kws1[w]
                    off = offs1[w]
                    hwin = wpool.tile([64, HID], BF16, tag="hwin")
                    nc.sync.dma_start(hwin[0:span, :], hloc[nb:nb + span, :])
                    srcw = epool.tile([P, k1max], I32, tag="srcw")
                    nc.sync.dma_start(srcw[:, 0:kw], src1_d[:, off:off + kw])
                    dstw = wpool.tile([P, k1max], F32, tag="dstw")
                    nc.sync.dma_start(dstw[:, 0:kw], dstl1_d[:, off:off + kw])
                    pre = epool.tile([P, k1max, P], BF16, tag="pre")
                    for c in range(kw):
                        nc.gpsimd.indirect_dma_start(
                            out=pre[:, c, 0:HID], out_offset=None,
                            in_=h_full[:, :],
                            in_offset=IndirectOffsetOnAxis(ap=srcw[:, c:c + 1], axis=0))
                    nc.sync.dma_start(
                        pre[:, 0:kw, HID:HID + ED],
                        ea1_d[:, off * ED:(off + kw) * ED].rearrange(
                            "p (k d) -> p k d", d=ED))
                    nc.vector.memset(pre[:, 0:kw, 127:P], 1.0)
                    i0, i1 = broadcast_tensor_aps(
                        iotaF[:, None, 0:w1], dstw[:, 0:kw, None])
                    nc.vector.tensor_tensor(
                        out=pre[:, 0:kw, 72:127], in0=i0, in1=i1, op=ALU.is_equal)
                    ptw = ps_t.tile([P, P], BF16, tag="ptp")
                    nc.tensor.transpose(ptw[0:HID, 0:span], hwin[0:span, :],
                                        ident[0:span, 0:span])
                    hwT = wpool.tile([HID, 64], BF16, tag="hwT")
                    nc.scalar.activation(hwT[:, 0:span], ptw[0:HID, 0:span], AF.Copy)
                    pxr = ps_p.tile([64, HC + H], F32, tag="pxp")
                    nc.tensor.matmul(pxr[0:span, :], lhsT=hwT[:, 0:span],
                                     rhs=wr1aug[:], start=True, stop=True)
                    rc1w = rc1ws[w % 2]
                    if w < 2:
                        nc.sync.dma_start(rc1w[:], rc1[:])
                    xrw = wpool.tile([64, HC + H], BF16, tag="xrw")
                    nc.scalar.activation(xrw[0:span, :], pxr[0:span, :], AF.Copy)
                    nc.sync.dma_start(rc1w[72:72 + span, :], xrw[0:span, :])
                    return pre, rc1w

                def chunks1(w, pre, rc1w):
                    span = min(w1, npc - w * w1)
                    nb = w * w1
                    kw = kws1[w]
                    stk = epool.tile([P, k1max, P], BF16, tag="stk")
                    for c in range(kw):
                        ptc = ps_t.tile([P, P], BF16, tag="ptp")
                        nc.tensor.transpose(ptc[:], pre[:, c, :], ident[:])
                        nc.scalar.activation(stk[:, c, :], ptc[:], AF.Copy)

                    pout = ps_o.tile([64, HC + H], F32, tag="pout")

                    def emit_pay1(c, psx, ex):
                        pay = kpool.tile([P, HC + H], BF16, tag="pay")
                        x0, x1 = broadcast_tensor_aps(
                            psx[:, :].rearrange("p (h x) -> p h x", x=65),
                            ex[:, :, None])
                        nc.vector.tensor_tensor(
                            out=pay[:, :].rearrange("p (h x) -> p h x", x=65),
                            in0=x0, in1=x1, op=ALU.mult)
                        nc.tensor.matmul(pout[0:span, :],
                                         lhsT=pre[:, c, 72:72 + span],
                                         rhs=pay[:], start=(c == 0),
                                         stop=(c == kw - 1))

                    pending = None
                    for c in range(kw):
                        pss = ps_s.tile([P, HC + H], F32, tag="pss")
                        nc.tensor.matmul(pss[:], lhsT=stk[:, c, :], rhs=rc1w[:],
                                         start=True, stop=True)
                        psx = ps_x.tile([P, HC + H], F32, tag="psx")
                        nc.tensor.matmul(psx[:], lhsT=stk[:, c, :], rhs=rc2[:],
                                         start=True, stop=True)
                        m = kpool.tile([P, HC], BF16, tag="m")
                        nc.vector.scalar_tensor_tensor(
                            out=m[:], in0=pss[:, 0:HC], scalar=0.0,
                            in1=attw1[:], op0=ALU.max, op1=ALU.mult)
                        alph = kpool.tile([P, H], F32, tag="alph")
                        nc.vector.tensor_reduce(
                            out=alph[:, :, None],
                            in_=m[:, :].rearrange("p (h x) -> p h x", x=HID),
                            axis=mybir.AxisListType.X, op=ALU.add)
                        alph2 = kpool.tile([P, H], F32, tag="alph2")
                        nc.vector.tensor_tensor(
                            out=alph2[:], in0=alph[:], in1=pss[:, HC:HC + H],
                            op=ALU.add)
                        ex = kpool.tile([P, H], F32, tag="ex")
                        nc.scalar.activation(ex[:], alph2[:], AF.Exp, bias=neg4[:])
                        if pending is not None:
                            emit_pay1(*pending)
                        pending = (c, psx, ex)
                    emit_pay1(*pending)

                    # normalize + relu -> h1 window; prep xl2 rows
                    pog = pout[0:span, :].rearrange("p (h x) -> p h x", x=65)
                    deng = wpool.tile([64, H], F32, tag="deng")
                    nc.vector.tensor_scalar(
                        out=deng[0:span, :, None], in0=pog[:, :, 64:65],
                        scalar1=1e-30, scalar2=None, op0=ALU.max)
                    rden = wpool.tile([64, H], F32, tag="rden")
                    nc.vector.reciprocal(rden[0:span, :], deng[0:span, :])
                    h1w = wpool.tile([64, HC], BF16, tag="h1w")
                    if span < 64:
                        nc.vector.memset(h1w[:], 0.0)
                    r0, r1 = broadcast_tensor_aps(
                        pog[:, :, 0:64], rden[0:span, :, None])
                    nc.vector.scalar_tensor_tensor(
                        out=h1w[0:span, :].rearrange("p (h x) -> p h x", x=HID),
                        in0=r0, scalar=0.0, in1=r1, op0=ALU.max, op1=ALU.mult)
                    nc.sync.dma_start(h1loc[nb:nb + span, :], h1w[0:span, :])

                    h1T = wpool.tile([P, 2, 64], BF16, tag="h1T")
                    for j in range(2):
                        ptj = ps_t.tile([P, P], BF16, tag="ptp")
                        nc.tensor.transpose(ptj[:, 0:span],
                                            h1w[0:span, j * P:(j + 1) * P],
                                            ident[0:span, 0:span])
                        nc.scalar.activation(h1T[:, j, 0:span], ptj[:, 0:span],
                                             AF.Copy)
                    pxl2 = ps_p.tile([64, HC + H], F32, tag="pxp")
                    nc.tensor.matmul(pxl2[0:span, 0:HC + 1], lhsT=h1T[:, 0, 0:span],
                                     rhs=wl2a[:], start=True, stop=False)
                    nc.tensor.matmul(pxl2[0:span, 0:HC + 1], lhsT=h1T[:, 1, 0:span],
                                     rhs=wl2b[:], start=False, stop=False)
                    nc.tensor.matmul(pxl2[0:span, 0:HC + 1], lhsT=ones1[:, 0:span],
                                     rhs=xl2bias[:], start=False, stop=True)
                    xl2w = wpool.tile([64, XW], BF16, tag="xl2w")
                    nc.scalar.activation(xl2w[0:span, 0:HC], pxl2[0:span, 0:HC],
                                         AF.Copy)
                    nc.vector.tensor_copy(
                        xl2w[0:span, HC:HC + 2].bitcast(F32),
                        pxl2[0:span, HC:HC + 1])
                    nc.vector.memset(xl2w[0:span, HC + 2:HC + 3], 1.0)
                    nc.vector.memset(xl2w[0:span, HC + 3:XW], 0.0)
                    nc.sync.dma_start(xl2loc[nb:nb + span, :], xl2w[0:span, :])

                state1 = prep1(0)
                for w in range(nw1):
                    nxt = prep1(w + 1) if w + 1 < nw1 else None
                    chunks1(w, *state1)
                    state1 = nxt

            # ---------------- phase 2: allgather xl2 table -------------------
            nc.gpsimd.collective_compute(
                "AllGather", ALU.bypass,
                replica_groups=[list(range(NCORES))],
                ins=[xl2loc[:]], outs=[xl2full[:]])

            # ---------------- phase 3: GAT layer 2 ---------------------------
            with (
                tc.tile_pool(name="w2p", bufs=2) as wpool,
                tc.tile_pool(name="e2p", bufs=3) as epool,
                tc.tile_pool(name="kp2", bufs=3) as kpool,
                tc.tile_pool(name="ps2_s", bufs=2, space="PSUM") as ps2_s,
                tc.tile_pool(name="ps2_t", bufs=2, space="PSUM") as ps2_t,
                tc.tile_pool(name="ps2_o", bufs=1, space="PSUM") as ps2_o,
                tc.tile_pool(name="ps2_p", bufs=1, space="PSUM") as ps2_p,
            ):
                def prep2(w):
                    span = min(w2, npc - w * w2)
                    nb = w * w2
                    kw = kws2[w]
                    off = offs2[w]
                    h1r = wpool.tile([P, HC], BF16, tag="h1r")
                    if span < P:
                        nc.vector.memset(h1r[:], 0.0)
                    nc.sync.dma_start(h1r[0:span, :], h1loc[nb:nb + span, :])
                    srcw2 = epool.tile([P, k2max], I32, tag="srcw2")
                    nc.sync.dma_start(srcw2[:, 0:kw], src2_d[:, off:off + kw])
                    xg = epool.tile([P, k2max, XW], BF16, tag="xg")
                    for c in range(kw):
                        nc.gpsimd.indirect_dma_start(
                            out=xg[:, c, :], out_offset=None,
                            in_=xl2full[:, :],
                            in_offset=IndirectOffsetOnAxis(ap=srcw2[:, c:c + 1], axis=0))
                    dstw2 = wpool.tile([P, k2max], F32, tag="dstw2")
                    nc.sync.dma_start(dstw2[:, 0:kw], dstl2_d[:, off:off + kw])
                    pre2 = epool.tile([P, k2max, P], BF16, tag="pre2")
                    nc.sync.dma_start(
                        pre2[:, 0:kw, 0:ED],
                        ea2_d[:, off * ED:(off + kw) * ED].rearrange(
                            "p (k d) -> p k d", d=ED))
                    nc.vector.memset(pre2[:, 0:kw, 127:P], 1.0)
                    i0, i1 = broadcast_tensor_aps(
                        iotaF[:, None, 0:w2], dstw2[:, 0:kw, None])
                    nc.vector.tensor_tensor(
                        out=pre2[:, 0:kw, ED:ED + w2], in0=i0, in1=i1,
                        op=ALU.is_equal)
                    h1rT = wpool.tile([P, 2, P], BF16, tag="h1rT")
                    for j in range(2):
                        ptj = ps2_t.tile([P, P], BF16, tag="ptp2")
                        nc.tensor.transpose(ptj[:, 0:span],
                                            h1r[0:span, j * P:(j + 1) * P],
                                            ident[0:span, 0:span])
                        nc.scalar.activation(h1rT[:, j, 0:span], ptj[:, 0:span],
                                             AF.Copy)
                    pxr2 = ps2_p.tile([P, HC + 1], F32, tag="pxr2")
                    nc.tensor.matmul(pxr2[0:span, :], lhsT=h1rT[:, 0, 0:span],
                                     rhs=wr2a[:], start=True, stop=False)
                    nc.tensor.matmul(pxr2[0:span, :], lhsT=h1rT[:, 1, 0:span],
                                     rhs=wr2b[:], start=False, stop=True)
                    rc21w = rc21ws[w % 2]
                    if w < 2:
                        nc.sync.dma_start(rc21w[:], rc21[:])
                    xrw2 = wpool.tile([P, HC + 1], BF16, tag="xrw2")
                    nc.scalar.activation(xrw2[0:span, :], pxr2[0:span, :], AF.Copy)
                    nc.sync.dma_start(rc21w[ED:ED + span, :], xrw2[0:span, :])
                    return pre2, xg, rc21w

                def chunks2(w, pre2, xg, rc21w):
                    span = min(w2, npc - w * w2)
                    nb = w * w2
                    kw = kws2[w]
                    ng = _ceil_div(kw, 2)
                    stk2 = epool.tile([P, k2max, P], BF16, tag="stk2")
                    for c in range(kw):
                        ptc = ps2_t.tile([P, P], BF16, tag="ptp2")
                        nc.tensor.transpose(ptc[:], pre2[:, c, :], ident[:])
                        nc.scalar.activation(stk2[:, c, :], ptc[:], AF.Copy)

                    pout2 = ps2_o.tile([P, HC + 3], F32, tag="pout2")

                    def emit_pay2(c0, gw, ex2):
                        pay2 = kpool.tile([P, 2, HC + 3], BF16, tag="pay2")
                        for j in range(gw):
                            nc.vector.tensor_scalar(
                                out=pay2[:, j, :], in0=xg[:, c0 + j, 0:HC + 3],
                                scalar1=ex2[:, j:j + 1], scalar2=None,
                                op0=ALU.mult)
                        for j in range(gw):
                            c = c0 + j
                            nc.tensor.matmul(
                                pout2[0:span, :], lhsT=pre2[:, c, ED:ED + span],
                                rhs=pay2[:, j, :], start=(c == 0),
                                stop=(c == kw - 1))

                    pending = None
                    for g in range(ng):
                        c0 = 2 * g
                        gw = min(2, kw - c0)
                        pss2 = ps2_s.tile([P, 2, 512], F32, tag="pss2")
                        for j in range(gw):
                            nc.tensor.matmul(
                                pss2[:, j, 0:HC + 1], lhsT=stk2[:, c0 + j, :],
                                rhs=rc21w[:], start=True, stop=False)
                            nc.tensor.matmul(
                                pss2[:, j, 0:P], lhsT=ident[:],
                                rhs=xg[:, c0 + j, 0:P], start=False, stop=False)
                            nc.tensor.matmul(
                                pss2[:, j, P:HC], lhsT=ident[:],
                                rhs=xg[:, c0 + j, P:HC], start=False, stop=True)
                        m2 = kpool.tile([P, 2, HC], BF16, tag="m2")
                        a0, a1 = broadcast_tensor_aps(
                            pss2[:, 0:gw, 0:HC], attw2[:, None, :])
                        nc.vector.scalar_tensor_tensor(
                            out=m2[:, 0:gw, :], in0=a0, scalar=0.0,
                            in1=a1, op0=ALU.max, op1=ALU.mult)
                        al2 = kpool.tile([P, 2], F32, tag="al2")
                        nc.vector.tensor_reduce(
                            out=al2[:, 0:gw, None], in_=m2[:, 0:gw, :],
                            axis=mybir.AxisListType.X, op=ALU.add)
                        al2b = kpool.tile([P, 2], F32, tag="al2b")
                        nc.vector.tensor_tensor(
                            out=al2b[:, 0:gw, None], in0=al2[:, 0:gw, None],
                            in1=pss2[:, 0:gw, HC:HC + 1], op=ALU.add)
                        al2c = kpool.tile([P, 2], F32, tag="al2c")
                        nc.vector.tensor_tensor(
                            out=al2c[:, 0:gw, None], in0=al2b[:, 0:gw, None],
                            in1=xg[:, c0:c0 + gw, HC:HC + 2].bitcast(F32),
                            op=ALU.add)
                        ex2 = kpool.tile([P, 2], F32, tag="ex2")
                        nc.scalar.activation(ex2[:, 0:gw], al2c[:, 0:gw], AF.Exp)
                        if pending is not None:
                            emit_pay2(*pending)
                        pending = (c0, gw, ex2)
                    emit_pay2(*pending)

                    deng2 = wpool.tile([P, 1], F32, tag="deng2")
                    nc.vector.tensor_scalar(
                        out=deng2[0:span, :], in0=pout2[0:span, HC + 2:HC + 3],
                        scalar1=1e-30, scalar2=None, op0=ALU.max)
                    rden2 = wpool.tile([P, 1], F32, tag="rden2")
                    nc.vector.reciprocal(rden2[0:span, :], deng2[0:span, :])
                    outw = wpool.tile([P, HC], F32, tag="outw")
                    b0, b1 = broadcast_tensor_aps(
                        pout2[0:span, 0:HC], rden2[0:span, :])
                    nc.vector.scalar_tensor_tensor(
                        out=outw[0:span, :], in0=b0, scalar=0.0,
                        in1=b1, op0=ALU.max, op1=ALU.mult)
                    nc.sync.dma_start(out_d[nb:nb + span, :], outw[0:span, :])

                state2 = prep2(0)
                for w in range(nw2):
                    nxt = prep2(w + 1) if w + 1 < nw2 else None
                    chunks2(w, *state2)
                    state2 = nxt

            if debug:
                nc.sync.dma_start(dbg_h[:, :], h_full[:, :])
                nc.sync.dma_start(dbg_h1[:, :], h1loc[:, :])
                nc.sync.dma_start(dbg_xf[:, :], xl2full[:, :])

    nc.finalize()
    return nc


# ----------------------------------------------------------------------------
# entry point
# ----------------------------------------------------------------------------

def _install_ntff_hook():
    """Shim antenv.axon_hooks so trace=True can collect NTFF profiles."""
    import types
    try:
        from antenv.axon_hooks import get_axon_ntff_profile_hook  # noqa: F401
        return
    except ImportError:
        pass
    try:
        import antenv
        boot_dir = "/root/.axon_site/trn_agent_boot"
        so_path = "/opt/axon/libaxon_pjrt.so"
        if boot_dir not in sys.path:
            sys.path.insert(0, boot_dir)
        import trn_boot
        mod = types.ModuleType("antenv.axon_hooks")
        _state = {"hook": None}
        mod.set_axon_ntff_profile_hook = lambda h: _state.__setitem__("hook", h)
        mod.get_axon_ntff_profile_hook = lambda: _state["hook"]
        sys.modules["antenv.axon_hooks"] = mod
        antenv.axon_hooks = mod
        if os.path.exists(so_path):
            mod.set_axon_ntff_profile_hook(
                trn_boot._ntff_profile_via_ctypes(so_path))
    except Exception as exc:  # profiling is best-effort
        print("ntff hook install failed:", exc)


def run(inputs, trace=False, debug=False):
    if trace:
        _install_ntff_hook()
    n = int(inputs["x"].shape[0])
    e = int(inputs["edge_index"].shape[1])
    assert n % NCORES == 0
    npc = n // NCORES
    meta, in_maps, perms = _prep_host(inputs, n, e, npc)
    nc = _build_nc(meta, debug=debug)
    res = run_bass_kernel_spmd(nc, in_maps, list(range(NCORES)), trace=trace)
    full = np.empty((n, HC), np.float32)
    for c in range(NCORES):
        full[c * npc + perms[c]] = res.results[c]["out"]
    return full, res


def kernel(**inputs):
    full, _ = run(inputs, trace=False)
    return full
